# revision 13
# baseline (speedup 1.0000x reference)
"""DualMemorySystem Trainium2 kernel — 8-core SPMD (batch x 4 row-bands).

Per core: one (b, 32-row out band). Convolution form of unfold/attention/fold:
  sim = conv(x, mem)      -> p matmuls per 4-row window, K=(kernel-row, c), fp16
  att = softmax_m(sim)    -> exp (ACT), ones-matmul partition sum,
                             reciprocal_approx (DVE) + multiplies (DVE+GpSimd)
  R_i = conv_x(att, mem)  -> matmuls over col-shifted att replicas, fp16
  out = fold_y(R)         -> log-tree of shifted adds (in-place), partition
                             moves via SBUF->SBUF DMA
  fusion: pooled partials via STT accum -> host MLP between launches ->
          tiny phase-B kernel applies softmax weights + combines.

Software-pipelined emission: branch order (p=7, p=5, p=3); per branch the
PE stream is conv1(k) [denominator matmuls trail at a 2-window lag], then
conv2(k-1), so the PE never waits on the current branch's softmax chain.
Double-buffered PSUM pools (8 banks exactly). DMAs ride only the sync and
gpsimd queues mid-kernel; large weight loads are deferred to just before
first use so the x8 image loads own the DMA engines at startup.

Hardware constraints baked in (probed): matmul dst partition base must be 0;
engines cannot remap partitions (only DMA/PE move data across partitions);
no divide ALU on DVE; fp32r matmul needs N>=256; DVE ops need 32-aligned
partition bases; only gpsimd DMAs may cast dtypes; DMA issue costs ~0.7us
of issuing-engine time regardless of size (so batch DMAs).
"""
import numpy as np
from contextlib import ExitStack

import concourse.bass as bass
import concourse.bacc as bacc
import concourse.tile as tile
from concourse import mybir
from concourse.bass_utils import run_bass_kernel_spmd

F32 = mybir.dt.float32
F32R = mybir.dt.float32r
F16 = mybir.dt.float16

B, C, H, W = 2, 16, 128, 128
PS = (3, 5, 7)
PADS = (1, 2, 3)
NBG, NTG = 64, 8
NCORES = 8
NBANDS = 4
BH = H // NBANDS            # 32 out rows per core
RX = 38                     # x replica rows per core
CX = 134                    # x cols with halo (128 + 6)
RA = 38                     # max att rows (32 + 2*padmax)
RAL = 40                    # R sbuf rows (fold-tree halo)
RAS = [BH + 2 * p for p in PADS]   # att rows per branch: 34, 36, 38
SEQ = (2, 1, 0)             # branch processing order: p=7, 5, 3
W2BASE = [0, 2, 5]

_CACHE = {}


def _windows(ra):
    return [(r0, min(4, ra - r0)) for r0 in range(0, ra, 4)]


def _build_A():
    nc = bacc.Bacc("TRN2", target_bir_lowering=False, debug=False,
                   num_devices=NCORES)

    d_x8bg = nc.dram_tensor("x8bg", [112, RX, CX], F16, kind="ExternalInput")
    d_x8tg = nc.dram_tensor("x8tg", [112, RX, CX], F16, kind="ExternalInput")
    d_hug = nc.dram_tensor("hug", [3, RA, W], F32, kind="ExternalInput")
    d_rdiv = nc.dram_tensor("rdiv", [96, BH, W], F16, kind="ExternalInput")
    d_ones = nc.dram_tensor("oneslhs", [73, 72], F32, kind="ExternalInput")
    d_w1all = nc.dram_tensor("w1all", [112, 1080], F16, kind="ExternalInput")
    d_w2bg = nc.dram_tensor("w2bg", [128, 9, 128], F16, kind="ExternalInput")
    d_w2tg = nc.dram_tensor("w2tg", [64, 3, 128], F16, kind="ExternalInput")
    d_fdiv = nc.dram_tensor("fdiv_out", [96, BH, W], F16,
                            kind="ExternalOutput")
    d_pool = nc.dram_tensor("pool_out", [32], F32, kind="ExternalOutput")

    with tile.TileContext(nc) as tc, ExitStack() as ctx:
        P = ctx.enter_context(tc.tile_pool(name="persist", bufs=1))
        pE = ctx.enter_context(tc.tile_pool(name="epool", bufs=2))
        pEtg = ctx.enter_context(tc.tile_pool(name="etg", bufs=2))
        pRcp = ctx.enter_context(tc.tile_pool(name="rcp", bufs=2))
        pS = ctx.enter_context(tc.tile_pool(name="spool", bufs=2))
        pR = ctx.enter_context(tc.tile_pool(name="rsb", bufs=4))
        pT = ctx.enter_context(tc.tile_pool(name="foldtmp", bufs=1))
        ps_c1bg = ctx.enter_context(
            tc.tile_pool(name="pc1bg", bufs=2, space=bass.MemorySpace.PSUM))
        ps_c1tg = ctx.enter_context(
            tc.tile_pool(name="pc1tg", bufs=2, space=bass.MemorySpace.PSUM))
        ps_den = ctx.enter_context(
            tc.tile_pool(name="pden", bufs=2, space=bass.MemorySpace.PSUM))
        ps_c2 = ctx.enter_context(
            tc.tile_pool(name="pc2", bufs=2, space=bass.MemorySpace.PSUM))

        # ---- startup-critical loads only: x8 images + first-branch weights
        x8 = {}
        t = P.tile([112, RX, CX], F16, tag="x8bg")
        nc.sync.dma_start(t[:, 0:20, :], d_x8bg[:, 0:20, :])
        nc.sync.dma_start(t[:, 20:RX, :], d_x8bg[:, 20:RX, :])
        x8["bg"] = t
        t = P.tile([112, RX, CX], F16, tag="x8tg")
        nc.gpsimd.dma_start(t[:, 0:20, :], d_x8tg[:, 0:20, :])
        nc.gpsimd.dma_start(t[:, 20:RX, :], d_x8tg[:, 20:RX, :])
        x8["tg"] = t
        w1all = P.tile([112, 1080], F16, tag="w1all")
        nc.scalar.dma_start(w1all[:], d_w1all[:])
        # packed col offsets: bg0,bg1,bg2 then tg0,tg1,tg2
        W1OFF = {("bg", 0): 0, ("bg", 1): 192, ("bg", 2): 512,
                 ("tg", 0): 960, ("tg", 1): 984, ("tg", 2): 1024}

        def w1ap(s, n, j):
            M = NBG if s == "bg" else NTG
            off = W1OFF[(s, n)] + j * M
            return w1all[0:16 * PS[n], off:off + M]

        ones_l = P.tile([73, 72], F32R, tag="ones")
        nc.gpsimd.dma_start(ones_l[:], d_ones[:])

        fdiv = P.tile([96, BH, W], F16, tag="fdiv")
        pacc = P.tile([96, 1], F32, tag="pacc")
        late = {}   # deferred persistent tiles: w2bg, w2tg, rdiv

        state = {}

        def load_late():
            t = P.tile([128, 9, 128], F16, tag="w2bg")
            nc.scalar.dma_start(t[:], d_w2bg[:])
            late["w2bg"] = t
            t = P.tile([64, 3, 128], F16, tag="w2tg")
            nc.scalar.dma_start(t[:], d_w2tg[:])
            late["w2tg"] = t
            t = P.tile([96, BH, W], F16, tag="rdiv")
            nc.scalar.dma_start(t[:], d_rdiv[:])
            late["rdiv"] = t

        def conv1(k):
            n = SEQ[k]
            p, pad, ra = PS[n], PADS[n], RAS[n]
            rxo = 6 - 2 * pad
            wins = _windows(ra)
            nw = len(wins)
            E = pE.tile([73, RA, W], F32R, tag="E")
            nc.gpsimd.dma_start(E[72:73, 0:ra, :], d_hug[n:n + 1, 0:ra, :])
            Sbg = pS.tile([128, RA, 136], F16, tag="Sbg")
            Stg = pS.tile([72, RA, 144], F16, tag="Stg")
            nc.gpsimd.memset(Sbg[:, :, 0:4], 0.0)
            nc.gpsimd.memset(Sbg[:, :, 131:136], 0.0)
            nc.gpsimd.memset(Stg[:, :, 0:15], 0.0)
            nc.gpsimd.memset(Stg[:, :, 136:144], 0.0)
            st_etg = {}

            def denom_pair(w0):
                # denominator + softmax muls for windows w0, w0+1 (<=8 rows)
                r0 = wins[w0][0]
                rr8 = wins[w0][1] + (wins[w0 + 1][1] if w0 + 1 < nw else 0)
                rcp = pRcp.tile([72, 8, W], F32, tag="rcp")
                segs = [(0, wins[w0][1])]
                if w0 + 1 < nw:
                    segs.append((wins[w0][1], wins[w0 + 1][1]))
                for h, rr in segs:
                    den = ps_den.tile([72, 4, W], F32, tag="den")
                    nc.tensor.matmul(den[0:72, 0:rr, :], ones_l[:, 0:72],
                                     E[:, r0 + h:r0 + h + rr, :],
                                     start=True, stop=True)
                    nc.vector.reciprocal_approx_fast(rcp[0:72, h:h + rr, :],
                                                     den[0:72, 0:rr, :])
                nc.vector.tensor_mul(Sbg[0:72, r0:r0 + rr8, 3:131],
                                     E[0:72, r0:r0 + rr8, :],
                                     rcp[0:72, 0:rr8, :])

            for w, (r0, rr) in enumerate(wins):
                st = ps_c1bg.tile([64, 4, W], F32, tag="c1bg")
                for j in range(p):
                    nc.tensor.matmul(
                        st[0:64, 0:rr, :],
                        w1ap("bg", n, j),
                        x8["bg"][0:16 * p, r0 + rxo:r0 + rxo + rr,
                                 j + 3 - pad:j + 3 - pad + W],
                        start=(j == 0), stop=(j == p - 1))
                nc.scalar.activation(E[0:64, r0:r0 + rr, :], st[0:64, 0:rr, :],
                                     mybir.ActivationFunctionType.Exp)
                stg = ps_c1tg.tile([8, 4, W], F32, tag="c1tg")
                for j in range(p):
                    nc.tensor.matmul(
                        stg[0:8, 0:rr, :],
                        w1ap("tg", n, j),
                        x8["tg"][0:16 * p, r0 + rxo:r0 + rxo + rr,
                                 j + 3 - pad:j + 3 - pad + W],
                        start=(j == 0), stop=(j == p - 1))
                if w % 2 == 0:
                    st_etg[w // 2] = pEtg.tile([8, 8, W], F32R, tag="etg", name="etg")
                etg = st_etg[w // 2]
                h = 0 if w % 2 == 0 else wins[w - 1][1]
                nc.scalar.activation(etg[0:8, h:h + rr, :], stg[0:8, 0:rr, :],
                                     mybir.ActivationFunctionType.Exp)
                if w % 2 == 1 or w == nw - 1:
                    w0 = w - (w % 2)
                    hh = wins[w0][1] + (rr if w % 2 == 1 else 0)
                    nc.gpsimd.dma_start(
                        E[64:72, wins[w0][0]:wins[w0][0] + hh, :],
                        etg[0:8, 0:hh, :])
                    if w0 >= 2:
                        denom_pair(w0 - 2)
                # deferred loads ride behind the first windows
                if k == 0 and w == 2:
                    load_late()
            # remaining denominator pair (in-loop covered up to last-2)
            denom_pair((nw - 1) - ((nw - 1) % 2))
            # full-branch replica DMAs; Stg replicas read the tg att out of
            # Sbg[64:72] BEFORE the Sbg self-replica overwrites that range
            for g in range(p):
                nc.sync.dma_start(Stg[8 * g:8 * g + 8, 0:ra, 8 + g:136 + g],
                                  Sbg[64:72, 0:ra, 3:131])
            # col 3 on partitions 64:72 held tg att; the self-replica below
            # only covers cols 4:132, so re-zero it once the reads are done
            nc.gpsimd.memset(Sbg[64:72, :, 3:4], 0.0)
            nc.sync.dma_start(Sbg[64:128, 0:ra, 4:132], Sbg[0:64, 0:ra, 3:131])
            state[k] = (Sbg, Stg)

        def conv2(k):
            n = SEQ[k]
            p, pad, ra = PS[n], PADS[n], RAS[n]
            Sbg, Stg = state[k]
            w2bg, w2tg = late["w2bg"], late["w2tg"]
            Rbg = pR.tile([128, RAL, W], F16, tag="R")
            Rtg = pR.tile([128, RAL, W], F16, tag="R")
            nc.gpsimd.memset(Rbg[:, ra:RAL, :], 0.0)
            nc.gpsimd.memset(Rtg[:, ra:RAL, :], 0.0)
            nchk = (p + 1) // 2
            for r0, rr in _windows(ra):
                rp = ps_c2.tile([128, 4, W], F32, tag="c2")
                for ci in range(nchk):
                    jj = 2 * ci
                    nc.tensor.matmul(
                        rp[:, 0:rr, :],
                        w2bg[:, W2BASE[n] + ci, :],
                        Sbg[:, r0:r0 + rr, 3 + pad - jj:3 + pad - jj + W],
                        start=(ci == 0), stop=(ci == nchk - 1))
                nc.scalar.activation(Rbg[:, r0:r0 + rr, :], rp[:, 0:rr, :],
                                     mybir.ActivationFunctionType.Copy)
                rp2 = ps_c2.tile([128, 4, W], F32, tag="c2")
                nc.tensor.matmul(rp2[0:128, 0:rr, :],
                                 w2tg[0:8 * p, n, :],
                                 Stg[0:8 * p, r0:r0 + rr, 8 + pad:8 + pad + W],
                                 start=True, stop=True)
                nc.scalar.activation(Rtg[:, r0:r0 + rr, :], rp2[:, 0:rr, :],
                                     mybir.ActivationFunctionType.Copy)
            state[k] = (Rbg, Rtg)

        def fold(k):
            n = SEQ[k]
            p = PS[n]
            Rs = state[k]
            T1 = {}
            T2 = {}
            T3 = {}
            ft = {}
            if p > 3:
                for si in range(2):
                    T1[si] = pT.tile([64, 35, W], F16, tag="T1", name=f"T1{si}")
                    nc.sync.dma_start(T1[si][0:64, 0:35, :],
                                      Rs[si][64:128, 4:39, :])
                for si in range(2):
                    nc.vector.tensor_add(Rs[si][0:64, 0:35, :],
                                         Rs[si][0:64, 0:35, :], T1[si][:])
            for si in range(2):
                T2[si] = pT.tile([32, 33, W], F16, tag="T2", name=f"T2{si}")
                nc.sync.dma_start(T2[si][0:32, 0:33, :], Rs[si][32:64, 2:35, :])
            for si in range(2):
                nc.vector.tensor_add(Rs[si][0:32, 0:33, :],
                                     Rs[si][0:32, 0:33, :], T2[si][:])
            for si in range(2):
                T3[si] = pT.tile([16, BH, W], F16, tag="T3", name=f"T3{si}")
                nc.sync.dma_start(T3[si][0:16, 0:BH, :],
                                  Rs[si][16:32, 1:BH + 1, :])
            for si in range(2):
                nc.vector.tensor_add(T3[si][:], Rs[si][0:16, 0:BH, :],
                                     T3[si][:])
                nc.gpsimd.dma_start(
                    fdiv[32 * n + 16 * si:32 * n + 16 * si + 16, :, :],
                    T3[si][:])

        # ---------------- pipelined emission ----------------
        conv1(0)
        conv1(1)
        conv2(0)
        conv1(2)
        fold(0)
        conv2(1)
        fold(1)
        conv2(2)
        fold(2)

        # fdiv = folded * rdiv (one pass), pooled partials, store
        nc.vector.scalar_tensor_tensor(
            fdiv[:], fdiv[:], 0.0, late["rdiv"][:],
            op0=mybir.AluOpType.bypass, op1=mybir.AluOpType.mult,
            accum_out=pacc[:])
        nc.sync.dma_start(d_fdiv[:], fdiv[:])

        # pooled partial combine -> pool_out
        pb = P.tile([32, 1], F32, tag="pb")
        pc = P.tile([32, 1], F32, tag="pc")
        nc.sync.dma_start(pb[:], pacc[32:64, :])
        nc.sync.dma_start(pc[:], pacc[64:96, :])
        pool32a = P.tile([32, 1], F32, tag="pool32a")
        pool32 = P.tile([32, 1], F32, tag="pool32")
        nc.vector.tensor_add(pool32a[:], pacc[0:32, :], pb[:])
        nc.vector.tensor_add(pool32[:], pool32a[:], pc[:])
        nc.sync.dma_start(d_pool[:], pool32[:, 0])

    nc.compile()
    return nc


def _build_B():
    nc = bacc.Bacc("TRN2", target_bir_lowering=False, debug=False,
                   num_devices=NCORES)
    d_f = nc.dram_tensor("fdiv_in", [96, BH, W], F16, kind="ExternalInput")
    d_wt = nc.dram_tensor("wt96", [96, 1], F32, kind="ExternalInput")
    d_obg = nc.dram_tensor("out_bg", [C, BH, W], F16, kind="ExternalOutput")
    d_otg = nc.dram_tensor("out_tg", [C, BH, W], F16, kind="ExternalOutput")

    with tile.TileContext(nc) as tc, ExitStack() as ctx:
        Q = ctx.enter_context(tc.tile_pool(name="q", bufs=1))
        fdv = Q.tile([96, BH, W], F16, tag="fdv")
        wt = Q.tile([96, 1], F32, tag="wt")
        nc.sync.dma_start(fdv[0:48, :, :], d_f[0:48, :, :])
        nc.scalar.dma_start(fdv[48:96, :, :], d_f[48:96, :, :])
        nc.sync.dma_start(wt[:], d_wt[:])
        nc.vector.tensor_scalar_mul(fdv[:], fdv[:], wt[:])
        gb = Q.tile([32, BH, W], F16, tag="gb")
        gc = Q.tile([32, BH, W], F16, tag="gc")
        nc.sync.dma_start(gb[:], fdv[32:64, :, :])
        nc.scalar.dma_start(gc[:], fdv[64:96, :, :])
        nc.vector.tensor_add(fdv[0:32, :, :], fdv[0:32, :, :], gb[:])
        nc.vector.tensor_add(fdv[0:32, :, :], fdv[0:32, :, :], gc[:])
        nc.sync.dma_start(d_obg[:], fdv[0:16, :, :])
        nc.sync.dma_start(d_otg[:], fdv[16:32, :, :])

    nc.compile()
    return nc


# ======================= host-side prep =======================

def _prep_core(inputs, b, k):
    y0 = BH * k
    m = {}
    for s, key in (("bg", "bg"), ("tg", "tg")):
        x = np.asarray(inputs[key])[b]          # [C, H, W]
        x8 = np.zeros((7, C, RX, CX), np.float32)
        for g in range(7):
            lo = y0 - 6 + g
            hi = lo + RX
            slo, shi = max(lo, 0), min(hi, H)
            if slo < shi:
                x8[g, :, slo - lo:shi - lo, 3:131] = x[:, slo:shi, :]
        m[f"x8{s}"] = x8.reshape(112, RX, CX).astype(np.float16)

    hug = np.zeros((3, RA, W), np.float32)
    for n, pad in enumerate(PADS):
        for r in range(RA):
            y = y0 - pad + r
            if not (0 <= y < H):
                hug[n, r, :] = 1e30
    m["hug"] = hug

    rdiv = np.zeros((96, BH, W), np.float32)
    for n, pad in enumerate(PADS):
        yy = np.arange(H)
        rc = np.minimum(yy, pad) + np.minimum(H - 1 - yy, pad) + 1.0
        cc = np.minimum(yy[:W], pad) + np.minimum(W - 1 - yy[:W], pad) + 1.0
        div = np.outer(rc[y0:y0 + BH], cc) + 1e-8
        r = (1.0 / div).astype(np.float32)
        for si in range(2):
            base = 32 * n + 16 * si
            rdiv[base:base + 16] = r[None, :, :]
    m["rdiv"] = rdiv.astype(np.float16)

    ones = np.zeros((73, 72), np.float32)
    ones[0:64, 0:64] = 1.0
    ones[64:72, 64:72] = 1.0
    ones[72, :] = 1.0
    m["oneslhs"] = ones

    w1all = np.zeros((112, 1080), np.float32)
    w1off = {("bg", 0): 0, ("bg", 1): 192, ("bg", 2): 512,
             ("tg", 0): 960, ("tg", 1): 984, ("tg", 2): 1024}
    for s, M, nmem in (("bg", NBG, "bg_mem"), ("tg", NTG, "tg_mem")):
        for n, p in enumerate(PS):
            mem = np.asarray(inputs[f"{nmem}{n}"])          # [M, C*p*p]
            temp = float(np.asarray(inputs[f"{s}_temp{n}"])[0])
            D = C * p * p
            arr = mem.reshape(M, C, p, p)
            w1 = arr.transpose(2, 1, 3, 0).reshape(p * C, p * M)
            off = w1off[(s, n)]
            w1all[0:16 * p, off:off + p * M] = w1 * (temp / np.sqrt(D))
    m["w1all"] = w1all.astype(np.float16)

    # fold consumes group q at row shift +q where q = 2*pad - i
    w2bg = np.zeros((2, NBG, 9, 8, 16), np.float32)
    for n, p in enumerate(PS):
        pad = PADS[n]
        arr = np.asarray(inputs[f"bg_mem{n}"]).reshape(NBG, C, p, p)
        for ci in range((p + 1) // 2):
            for g in range(2):
                j = 2 * ci + g
                if j < p:
                    for i in range(p):
                        w2bg[g, :, W2BASE[n] + ci, 2 * pad - i, :] = \
                            arr[:, :, i, j]
    m["w2bg"] = w2bg.reshape(128, 9, 128).astype(np.float16)

    w2tg = np.zeros((8, NTG, 3, 8, 16), np.float32)
    for n, p in enumerate(PS):
        pad = PADS[n]
        arr = np.asarray(inputs[f"tg_mem{n}"]).reshape(NTG, C, p, p)
        for g in range(p):
            for i in range(p):
                w2tg[g, :, n, 2 * pad - i, :] = arr[:, :, i, g]
    m["w2tg"] = w2tg.reshape(64, 3, 128).astype(np.float16)
    return m


def _host_mlp(inputs, poolsum):
    """Per batch: pooled -> relu MLP -> softmax over scales -> wt96."""
    wt96 = np.zeros((96, 1), np.float32)
    for si, s in enumerate(("bg", "tg")):
        pooled = poolsum[16 * si:16 * si + 16] / (H * W)
        w1 = np.asarray(inputs[f"{s}_fc1_w"], np.float64)
        b1 = np.asarray(inputs[f"{s}_fc1_b"], np.float64)
        w2 = np.asarray(inputs[f"{s}_fc2_w"], np.float64)
        b2 = np.asarray(inputs[f"{s}_fc2_b"], np.float64)
        hdn = np.maximum(w1 @ pooled + b1, 0.0)
        logits = (w2 @ hdn + b2).reshape(3, 16)
        e = np.exp(logits - logits.max(axis=0, keepdims=True))
        wt = e / e.sum(axis=0, keepdims=True)
        for n in range(3):
            wt96[32 * n + 16 * si:32 * n + 16 * si + 16, 0] = wt[n]
    return wt96


def kernel(**inputs):
    if "ncA" not in _CACHE:
        _CACHE["ncA"] = _build_A()
        _CACHE["ncB"] = _build_B()

    in_maps = []
    for core in range(NCORES):
        b, k = divmod(core, NBANDS)
        in_maps.append(_prep_core(inputs, b, k))

    resA = run_bass_kernel_spmd(_CACHE["ncA"], in_maps, list(range(NCORES)))

    # host glue: reduce pooled partials within each batch's 4-band group,
    # then the tiny fusion MLP (exact, fp64)
    wt = {}
    for b in range(B):
        poolsum = np.sum([resA.results[b * NBANDS + k]["pool_out"]
                          for k in range(NBANDS)], axis=0).astype(np.float64)
        wt[b] = _host_mlp(inputs, poolsum)

    in_mapsB = []
    for core in range(NCORES):
        b, k = divmod(core, NBANDS)
        in_mapsB.append({
            "fdiv_in": resA.results[core]["fdiv_out"],
            "wt96": wt[b],
        })
    resB = run_bass_kernel_spmd(_CACHE["ncB"], in_mapsB, list(range(NCORES)))

    f_bc = np.zeros((B, C, H, W), np.float32)
    f_tg = np.zeros((B, C, H, W), np.float32)
    for core in range(NCORES):
        b, k = divmod(core, NBANDS)
        y0 = BH * k
        f_bc[b, :, y0:y0 + BH, :] = resB.results[core]["out_bg"].astype(np.float32)
        f_tg[b, :, y0:y0 + BH, :] = resB.results[core]["out_tg"].astype(np.float32)
    return (f_bc, f_tg)


# revision 17
# speedup vs baseline: 1.0670x; 1.0670x over previous
"""DualMemorySystem Trainium2 kernel — 8-core SPMD (batch x 4 row-bands).

Per core: one (b, 32-row out band). Convolution form of unfold/attention/fold:
  sim = conv(x, mem)      -> p matmuls per 4-row window, K=(kernel-row, c), fp16
  att = softmax_m(sim)    -> exp (ACT), ones-matmul partition sum,
                             reciprocal_approx (DVE) + multiplies (DVE+GpSimd)
  R_i = conv_x(att, mem)  -> matmuls over col-shifted att replicas, fp16
  out = fold_y(R)         -> log-tree of shifted adds (in-place), partition
                             moves via SBUF->SBUF DMA
  fusion: pooled partials via STT accum -> host MLP between launches ->
          tiny phase-B kernel applies softmax weights + combines.

Software-pipelined emission: branch order (p=7, p=5, p=3); per branch the
PE stream is conv1(k) [denominator matmuls trail at a 2-window lag], then
conv2(k-1), so the PE never waits on the current branch's softmax chain.
Double-buffered PSUM pools (8 banks exactly). DMAs ride only the sync and
gpsimd queues mid-kernel; large weight loads are deferred to just before
first use so the x8 image loads own the DMA engines at startup.

Hardware constraints baked in (probed): matmul dst partition base must be 0;
engines cannot remap partitions (only DMA/PE move data across partitions);
no divide ALU on DVE; fp32r matmul needs N>=256; DVE ops need 32-aligned
partition bases; only gpsimd DMAs may cast dtypes; DMA issue costs ~0.7us
of issuing-engine time regardless of size (so batch DMAs).
"""
import numpy as np
from contextlib import ExitStack

import concourse.bass as bass
import concourse.bacc as bacc
import concourse.tile as tile
from concourse import mybir
from concourse.bass_utils import run_bass_kernel_spmd

F32 = mybir.dt.float32
F32R = mybir.dt.float32r
F16 = mybir.dt.float16

B, C, H, W = 2, 16, 128, 128
PS = (3, 5, 7)
PADS = (1, 2, 3)
NBG, NTG = 64, 8
NCORES = 8
NBANDS = 4
BH = H // NBANDS            # 32 out rows per core
RX = 38                     # x replica rows per core
CX = 134                    # x cols with halo (128 + 6)
RA = 38                     # max att rows (32 + 2*padmax)
RAL = 40                    # R sbuf rows (fold-tree halo)
RAS = [BH + 2 * p for p in PADS]   # att rows per branch: 34, 36, 38
SEQ = (2, 1, 0)             # branch processing order: p=7, 5, 3
W2BASE = [0, 2, 5]

_CACHE = {}


def _windows(ra):
    return [(r0, min(4, ra - r0)) for r0 in range(0, ra, 4)]


def _build_A():
    nc = bacc.Bacc("TRN2", target_bir_lowering=False, debug=False,
                   num_devices=NCORES)

    d_x8bg = nc.dram_tensor("x8bg", [112, RX, CX], F16, kind="ExternalInput")
    d_x8tg = nc.dram_tensor("x8tg", [112, RX, CX], F16, kind="ExternalInput")
    d_hug = nc.dram_tensor("hug", [3, RA, W], F32, kind="ExternalInput")
    d_rdiv = nc.dram_tensor("rdiv", [96, BH, W], F16, kind="ExternalInput")
    d_ones = nc.dram_tensor("oneslhs", [73, 72], F32, kind="ExternalInput")
    d_w1all = nc.dram_tensor("w1all", [112, 1080], F16, kind="ExternalInput")
    d_selw = nc.dram_tensor("selw", [128, 3, 16], F16, kind="ExternalInput")
    d_w2bg = nc.dram_tensor("w2bg", [128, 9, 128], F16, kind="ExternalInput")
    d_w2tg = nc.dram_tensor("w2tg", [64, 3, 128], F16, kind="ExternalInput")
    d_fdiv = nc.dram_tensor("fdiv_out", [96, BH, W], F16,
                            kind="ExternalOutput")
    d_pool = nc.dram_tensor("pool_out", [32], F32, kind="ExternalOutput")

    with tile.TileContext(nc) as tc, ExitStack() as ctx:
        P = ctx.enter_context(tc.tile_pool(name="persist", bufs=1))
        pE = ctx.enter_context(tc.tile_pool(name="epool", bufs=2))
        pEtg = ctx.enter_context(tc.tile_pool(name="etg", bufs=2))
        pRcp = ctx.enter_context(tc.tile_pool(name="rcp", bufs=2))
        pS = ctx.enter_context(tc.tile_pool(name="spool", bufs=2))
        pR = ctx.enter_context(tc.tile_pool(name="rsb", bufs=4))
        pQ = ctx.enter_context(tc.tile_pool(name="qpool", bufs=2))
        pQo = ctx.enter_context(tc.tile_pool(name="qout", bufs=1))
        ps_c1bg = ctx.enter_context(
            tc.tile_pool(name="pc1bg", bufs=2, space=bass.MemorySpace.PSUM))
        ps_c1tg = ctx.enter_context(
            tc.tile_pool(name="pc1tg", bufs=2, space=bass.MemorySpace.PSUM))
        ps_den = ctx.enter_context(
            tc.tile_pool(name="pden", bufs=2, space=bass.MemorySpace.PSUM))
        ps_c2 = ctx.enter_context(
            tc.tile_pool(name="pc2", bufs=2, space=bass.MemorySpace.PSUM))

        # ---- startup-critical loads only: x8 images + first-branch weights
        x8 = {}
        t = P.tile([112, RX, CX], F16, tag="x8bg")
        nc.sync.dma_start(t[:, 0:20, :], d_x8bg[:, 0:20, :])
        nc.sync.dma_start(t[:, 20:RX, :], d_x8bg[:, 20:RX, :])
        x8["bg"] = t
        t = P.tile([112, RX, CX], F16, tag="x8tg")
        nc.gpsimd.dma_start(t[:, 0:20, :], d_x8tg[:, 0:20, :])
        nc.gpsimd.dma_start(t[:, 20:RX, :], d_x8tg[:, 20:RX, :])
        x8["tg"] = t
        w1all = P.tile([112, 1080], F16, tag="w1all")
        nc.scalar.dma_start(w1all[:], d_w1all[:])
        # packed col offsets: bg0,bg1,bg2 then tg0,tg1,tg2
        W1OFF = {("bg", 0): 0, ("bg", 1): 192, ("bg", 2): 512,
                 ("tg", 0): 960, ("tg", 1): 984, ("tg", 2): 1024}

        def w1ap(s, n, j):
            M = NBG if s == "bg" else NTG
            off = W1OFF[(s, n)] + j * M
            return w1all[0:16 * PS[n], off:off + M]

        ones_l = P.tile([73, 72], F32R, tag="ones")
        nc.gpsimd.dma_start(ones_l[:], d_ones[:])

        fdiv = P.tile([96, BH, W], F16, tag="fdiv")
        pacc = P.tile([96, 1], F32, tag="pacc")
        late = {}   # deferred persistent tiles: w2bg, w2tg, rdiv

        state = {}

        def load_late():
            t = P.tile([128, 9, 128], F16, tag="w2bg")
            nc.scalar.dma_start(t[:], d_w2bg[:])
            late["w2bg"] = t
            t = P.tile([64, 3, 128], F16, tag="w2tg")
            nc.scalar.dma_start(t[:], d_w2tg[:])
            late["w2tg"] = t
            t = P.tile([96, BH, W], F16, tag="rdiv")
            nc.scalar.dma_start(t[:], d_rdiv[:])
            late["rdiv"] = t
            t = P.tile([128, 3, 16], F16, tag="selw")
            nc.scalar.dma_start(t[:], d_selw[:])
            late["selw"] = t

        def conv1(k):
            n = SEQ[k]
            p, pad, ra = PS[n], PADS[n], RAS[n]
            rxo = 6 - 2 * pad
            wins = _windows(ra)
            nw = len(wins)
            E = pE.tile([73, RA, W], F32R, tag="E")
            nc.gpsimd.dma_start(E[72:73, 0:ra, :], d_hug[n:n + 1, 0:ra, :])
            Sbg = pS.tile([128, RA, 136], F16, tag="Sbg")
            Stg = pS.tile([72, RA, 144], F16, tag="Stg")
            nc.gpsimd.memset(Sbg[:, :, 0:4], 0.0)
            nc.gpsimd.memset(Sbg[:, :, 131:136], 0.0)
            nc.gpsimd.memset(Stg[:, :, 0:15], 0.0)
            nc.gpsimd.memset(Stg[:, :, 136:144], 0.0)
            st_etg = {}

            def denom_pair(w0):
                # denominator + softmax muls for windows w0, w0+1 (<=8 rows)
                r0 = wins[w0][0]
                rr8 = wins[w0][1] + (wins[w0 + 1][1] if w0 + 1 < nw else 0)
                rcp = pRcp.tile([72, 8, W], F32, tag="rcp")
                segs = [(0, wins[w0][1])]
                if w0 + 1 < nw:
                    segs.append((wins[w0][1], wins[w0 + 1][1]))
                for h, rr in segs:
                    den = ps_den.tile([72, 4, W], F32, tag="den")
                    nc.tensor.matmul(den[0:72, 0:rr, :], ones_l[:, 0:72],
                                     E[:, r0 + h:r0 + h + rr, :],
                                     start=True, stop=True)
                    nc.vector.reciprocal_approx_fast(rcp[0:72, h:h + rr, :],
                                                     den[0:72, 0:rr, :])
                nc.vector.tensor_mul(Sbg[0:64, r0:r0 + rr8, 3:131],
                                     E[0:64, r0:r0 + rr8, :],
                                     rcp[0:64, 0:rr8, :])
                nc.gpsimd.tensor_mul(Stg[64:72, r0:r0 + rr8, 8:136],
                                     E[64:72, r0:r0 + rr8, :],
                                     rcp[64:72, 0:rr8, :])

            for w, (r0, rr) in enumerate(wins):
                st = ps_c1bg.tile([64, 4, W], F32, tag="c1bg")
                for j in range(p):
                    nc.tensor.matmul(
                        st[0:64, 0:rr, :],
                        w1ap("bg", n, j),
                        x8["bg"][0:16 * p, r0 + rxo:r0 + rxo + rr,
                                 j + 3 - pad:j + 3 - pad + W],
                        start=(j == 0), stop=(j == p - 1))
                nc.scalar.activation(E[0:64, r0:r0 + rr, :], st[0:64, 0:rr, :],
                                     mybir.ActivationFunctionType.Exp)
                stg = ps_c1tg.tile([8, 4, W], F32, tag="c1tg")
                for j in range(p):
                    nc.tensor.matmul(
                        stg[0:8, 0:rr, :],
                        w1ap("tg", n, j),
                        x8["tg"][0:16 * p, r0 + rxo:r0 + rxo + rr,
                                 j + 3 - pad:j + 3 - pad + W],
                        start=(j == 0), stop=(j == p - 1))
                if w % 2 == 0:
                    st_etg[w // 2] = pEtg.tile([8, 8, W], F32R, tag="etg", name="etg")
                etg = st_etg[w // 2]
                h = 0 if w % 2 == 0 else wins[w - 1][1]
                nc.scalar.activation(etg[0:8, h:h + rr, :], stg[0:8, 0:rr, :],
                                     mybir.ActivationFunctionType.Exp)
                if w % 2 == 1 or w == nw - 1:
                    w0 = w - (w % 2)
                    hh = wins[w0][1] + (rr if w % 2 == 1 else 0)
                    nc.gpsimd.dma_start(
                        E[64:72, wins[w0][0]:wins[w0][0] + hh, :],
                        etg[0:8, 0:hh, :])
                    if w0 >= 2:
                        denom_pair(w0 - 2)
                # deferred loads ride behind the first windows
                if k == 0 and w == 2:
                    load_late()
            # remaining denominator pair (in-loop covered up to last-2)
            denom_pair((nw - 1) - ((nw - 1) % 2))
            # full-branch replica DMAs
            for g in range(p):
                nc.sync.dma_start(Stg[8 * g:8 * g + 8, 0:ra, 8 + g:136 + g],
                                  Stg[64:72, 0:ra, 8:136])
            nc.sync.dma_start(Sbg[64:128, 0:ra, 4:132], Sbg[0:64, 0:ra, 3:131])
            state[k] = (Sbg, Stg)

        def conv2(k):
            n = SEQ[k]
            p, pad, ra = PS[n], PADS[n], RAS[n]
            Sbg, Stg = state[k]
            w2bg, w2tg = late["w2bg"], late["w2tg"]
            Rbg = pR.tile([128, RA, W], F16, tag="R")
            Rtg = pR.tile([128, RA, W], F16, tag="R")
            nchk = (p + 1) // 2
            for r0, rr in _windows(ra):
                rp = ps_c2.tile([128, 4, W], F32, tag="c2")
                for ci in range(nchk):
                    jj = 2 * ci
                    nc.tensor.matmul(
                        rp[:, 0:rr, :],
                        w2bg[:, W2BASE[n] + ci, :],
                        Sbg[:, r0:r0 + rr, 3 + pad - jj:3 + pad - jj + W],
                        start=(ci == 0), stop=(ci == nchk - 1))
                nc.scalar.activation(Rbg[:, r0:r0 + rr, :], rp[:, 0:rr, :],
                                     mybir.ActivationFunctionType.Copy)
                rp2 = ps_c2.tile([128, 4, W], F32, tag="c2")
                nc.tensor.matmul(rp2[0:128, 0:rr, :],
                                 w2tg[0:8 * p, n, :],
                                 Stg[0:8 * p, r0:r0 + rr, 8 + pad:8 + pad + W],
                                 start=True, stop=True)
                nc.scalar.activation(Rtg[:, r0:r0 + rr, :], rp2[:, 0:rr, :],
                                     mybir.ActivationFunctionType.Copy)
            state[k] = (Rbg, Rtg)

        def fold(k):
            # fold_y on the PE: align each group's rows with per-group DMAs
            # (only DMAs can shift rows per partition group), then contract
            # the groups with a 0/1 selection matrix (K=16p, M=16).
            n = SEQ[k]
            p = PS[n]
            Rs = state[k]
            sel = late["selw"]
            Q = {}
            Qo = {}
            for si in range(2):
                Q[si] = pQ.tile([128, BH, W], F16, tag="Q", name=f"Q{si}")
                for g in range(p):
                    nc.sync.dma_start(Q[si][16 * g:16 * g + 16, :, :],
                                      Rs[si][16 * g:16 * g + 16, g:g + BH, :])
                Qo[si] = pQo.tile([16, BH, W], F16, tag="Qo", name=f"Qo{si}")
            for si in range(2):
                for r0 in range(0, BH, 4):
                    rpf = ps_c2.tile([16, 4, W], F32, tag="c2", name="rpf")
                    nc.tensor.matmul(rpf[0:16, :, :], sel[0:16 * p, n, :],
                                     Q[si][0:16 * p, r0:r0 + 4, :],
                                     start=True, stop=True)
                    nc.vector.tensor_copy(Qo[si][0:16, r0:r0 + 4, :],
                                          rpf[0:16, :, :])
                nc.gpsimd.dma_start(
                    fdiv[32 * n + 16 * si:32 * n + 16 * si + 16, :, :],
                    Qo[si][:])

        # ---------------- pipelined emission ----------------
        conv1(0)
        conv1(1)
        conv2(0)
        conv1(2)
        fold(0)
        conv2(1)
        fold(1)
        conv2(2)
        fold(2)

        # fdiv = folded * rdiv (one pass), pooled partials, store
        nc.vector.scalar_tensor_tensor(
            fdiv[:], fdiv[:], 0.0, late["rdiv"][:],
            op0=mybir.AluOpType.bypass, op1=mybir.AluOpType.mult,
            accum_out=pacc[:])
        nc.sync.dma_start(d_fdiv[:], fdiv[:])

        # pooled partial combine -> pool_out
        pb = P.tile([32, 1], F32, tag="pb")
        pc = P.tile([32, 1], F32, tag="pc")
        nc.sync.dma_start(pb[:], pacc[32:64, :])
        nc.sync.dma_start(pc[:], pacc[64:96, :])
        pool32a = P.tile([32, 1], F32, tag="pool32a")
        pool32 = P.tile([32, 1], F32, tag="pool32")
        nc.vector.tensor_add(pool32a[:], pacc[0:32, :], pb[:])
        nc.vector.tensor_add(pool32[:], pool32a[:], pc[:])
        nc.sync.dma_start(d_pool[:], pool32[:, 0])

    nc.compile()
    return nc


def _build_B():
    nc = bacc.Bacc("TRN2", target_bir_lowering=False, debug=False,
                   num_devices=NCORES)
    d_f = nc.dram_tensor("fdiv_in", [96, BH, W], F16, kind="ExternalInput")
    d_wt = nc.dram_tensor("wt96", [96, 1], F32, kind="ExternalInput")
    d_obg = nc.dram_tensor("out_bg", [C, BH, W], F16, kind="ExternalOutput")
    d_otg = nc.dram_tensor("out_tg", [C, BH, W], F16, kind="ExternalOutput")

    with tile.TileContext(nc) as tc, ExitStack() as ctx:
        Q = ctx.enter_context(tc.tile_pool(name="q", bufs=1))
        fdv = Q.tile([96, BH, W], F16, tag="fdv")
        wt = Q.tile([96, 1], F32, tag="wt")
        nc.sync.dma_start(fdv[0:48, :, :], d_f[0:48, :, :])
        nc.scalar.dma_start(fdv[48:96, :, :], d_f[48:96, :, :])
        nc.sync.dma_start(wt[:], d_wt[:])
        nc.vector.tensor_scalar_mul(fdv[:], fdv[:], wt[:])
        gb = Q.tile([32, BH, W], F16, tag="gb")
        gc = Q.tile([32, BH, W], F16, tag="gc")
        nc.sync.dma_start(gb[:], fdv[32:64, :, :])
        nc.scalar.dma_start(gc[:], fdv[64:96, :, :])
        nc.vector.tensor_add(fdv[0:32, :, :], fdv[0:32, :, :], gb[:])
        nc.vector.tensor_add(fdv[0:32, :, :], fdv[0:32, :, :], gc[:])
        nc.sync.dma_start(d_obg[:], fdv[0:16, :, :])
        nc.sync.dma_start(d_otg[:], fdv[16:32, :, :])

    nc.compile()
    return nc


# ======================= host-side prep =======================

def _prep_core(inputs, b, k):
    y0 = BH * k
    m = {}
    for s, key in (("bg", "bg"), ("tg", "tg")):
        x = np.asarray(inputs[key])[b]          # [C, H, W]
        x8 = np.zeros((7, C, RX, CX), np.float32)
        for g in range(7):
            lo = y0 - 6 + g
            hi = lo + RX
            slo, shi = max(lo, 0), min(hi, H)
            if slo < shi:
                x8[g, :, slo - lo:shi - lo, 3:131] = x[:, slo:shi, :]
        m[f"x8{s}"] = x8.reshape(112, RX, CX).astype(np.float16)

    hug = np.zeros((3, RA, W), np.float32)
    for n, pad in enumerate(PADS):
        for r in range(RA):
            y = y0 - pad + r
            if not (0 <= y < H):
                hug[n, r, :] = 1e30
    m["hug"] = hug

    rdiv = np.zeros((96, BH, W), np.float32)
    for n, pad in enumerate(PADS):
        yy = np.arange(H)
        rc = np.minimum(yy, pad) + np.minimum(H - 1 - yy, pad) + 1.0
        cc = np.minimum(yy[:W], pad) + np.minimum(W - 1 - yy[:W], pad) + 1.0
        div = np.outer(rc[y0:y0 + BH], cc) + 1e-8
        r = (1.0 / div).astype(np.float32)
        for si in range(2):
            base = 32 * n + 16 * si
            rdiv[base:base + 16] = r[None, :, :]
    m["rdiv"] = rdiv.astype(np.float16)

    ones = np.zeros((73, 72), np.float32)
    ones[0:64, 0:64] = 1.0
    ones[64:72, 64:72] = 1.0
    ones[72, :] = 1.0
    m["oneslhs"] = ones

    w1all = np.zeros((112, 1080), np.float32)
    w1off = {("bg", 0): 0, ("bg", 1): 192, ("bg", 2): 512,
             ("tg", 0): 960, ("tg", 1): 984, ("tg", 2): 1024}
    for s, M, nmem in (("bg", NBG, "bg_mem"), ("tg", NTG, "tg_mem")):
        for n, p in enumerate(PS):
            mem = np.asarray(inputs[f"{nmem}{n}"])          # [M, C*p*p]
            temp = float(np.asarray(inputs[f"{s}_temp{n}"])[0])
            D = C * p * p
            arr = mem.reshape(M, C, p, p)
            w1 = arr.transpose(2, 1, 3, 0).reshape(p * C, p * M)
            off = w1off[(s, n)]
            w1all[0:16 * p, off:off + p * M] = w1 * (temp / np.sqrt(D))
    m["w1all"] = w1all.astype(np.float16)

    # fold consumes group q at row shift +q where q = 2*pad - i
    w2bg = np.zeros((2, NBG, 9, 8, 16), np.float32)
    for n, p in enumerate(PS):
        pad = PADS[n]
        arr = np.asarray(inputs[f"bg_mem{n}"]).reshape(NBG, C, p, p)
        for ci in range((p + 1) // 2):
            for g in range(2):
                j = 2 * ci + g
                if j < p:
                    for i in range(p):
                        w2bg[g, :, W2BASE[n] + ci, 2 * pad - i, :] = \
                            arr[:, :, i, j]
    m["w2bg"] = w2bg.reshape(128, 9, 128).astype(np.float16)

    w2tg = np.zeros((8, NTG, 3, 8, 16), np.float32)
    for n, p in enumerate(PS):
        pad = PADS[n]
        arr = np.asarray(inputs[f"tg_mem{n}"]).reshape(NTG, C, p, p)
        for g in range(p):
            for i in range(p):
                w2tg[g, :, n, 2 * pad - i, :] = arr[:, :, i, g]
    m["w2tg"] = w2tg.reshape(64, 3, 128).astype(np.float16)

    selw = np.zeros((128, 3, 16), np.float32)
    for n, p in enumerate(PS):
        for g in range(p):
            for c in range(16):
                selw[16 * g + c, n, c] = 1.0
    m["selw"] = selw.astype(np.float16)
    return m


def _host_mlp(inputs, poolsum):
    """Per batch: pooled -> relu MLP -> softmax over scales -> wt96."""
    wt96 = np.zeros((96, 1), np.float32)
    for si, s in enumerate(("bg", "tg")):
        pooled = poolsum[16 * si:16 * si + 16] / (H * W)
        w1 = np.asarray(inputs[f"{s}_fc1_w"], np.float64)
        b1 = np.asarray(inputs[f"{s}_fc1_b"], np.float64)
        w2 = np.asarray(inputs[f"{s}_fc2_w"], np.float64)
        b2 = np.asarray(inputs[f"{s}_fc2_b"], np.float64)
        hdn = np.maximum(w1 @ pooled + b1, 0.0)
        logits = (w2 @ hdn + b2).reshape(3, 16)
        e = np.exp(logits - logits.max(axis=0, keepdims=True))
        wt = e / e.sum(axis=0, keepdims=True)
        for n in range(3):
            wt96[32 * n + 16 * si:32 * n + 16 * si + 16, 0] = wt[n]
    return wt96


def kernel(**inputs):
    if "ncA" not in _CACHE:
        _CACHE["ncA"] = _build_A()
        _CACHE["ncB"] = _build_B()

    in_maps = []
    for core in range(NCORES):
        b, k = divmod(core, NBANDS)
        in_maps.append(_prep_core(inputs, b, k))

    resA = run_bass_kernel_spmd(_CACHE["ncA"], in_maps, list(range(NCORES)))

    # host glue: reduce pooled partials within each batch's 4-band group,
    # then the tiny fusion MLP (exact, fp64)
    wt = {}
    for b in range(B):
        poolsum = np.sum([resA.results[b * NBANDS + k]["pool_out"]
                          for k in range(NBANDS)], axis=0).astype(np.float64)
        wt[b] = _host_mlp(inputs, poolsum)

    in_mapsB = []
    for core in range(NCORES):
        b, k = divmod(core, NBANDS)
        in_mapsB.append({
            "fdiv_in": resA.results[core]["fdiv_out"],
            "wt96": wt[b],
        })
    resB = run_bass_kernel_spmd(_CACHE["ncB"], in_mapsB, list(range(NCORES)))

    f_bc = np.zeros((B, C, H, W), np.float32)
    f_tg = np.zeros((B, C, H, W), np.float32)
    for core in range(NCORES):
        b, k = divmod(core, NBANDS)
        y0 = BH * k
        f_bc[b, :, y0:y0 + BH, :] = resB.results[core]["out_bg"].astype(np.float32)
        f_tg[b, :, y0:y0 + BH, :] = resB.results[core]["out_tg"].astype(np.float32)
    return (f_bc, f_tg)


# revision 20
# speedup vs baseline: 1.1960x; 1.1209x over previous
"""DualMemorySystem Trainium2 kernel — 8-core SPMD (batch x 4 row-bands).

Per core: one (b, 32-row out band). Convolution form of unfold/attention/fold:
  sim = conv(x, mem)      -> p matmuls per 4-row window, K=(kernel-row, c), fp16
  att = softmax_m(sim)    -> exp (ACT), ones-matmul partition sum,
                             reciprocal_approx (DVE) + multiplies (DVE+GpSimd)
  R_i = conv_x(att, mem)  -> matmuls over col-shifted att replicas, fp16
  out = fold_y(R)         -> log-tree of shifted adds (in-place), partition
                             moves via SBUF->SBUF DMA
  fusion: pooled partials via STT accum -> host MLP between launches ->
          tiny phase-B kernel applies softmax weights + combines.

Software-pipelined emission: branch order (p=7, p=5, p=3); per branch the
PE stream is conv1(k) [denominator matmuls trail at a 2-window lag], then
conv2(k-1), so the PE never waits on the current branch's softmax chain.
Double-buffered PSUM pools (8 banks exactly). DMAs ride only the sync and
gpsimd queues mid-kernel; large weight loads are deferred to just before
first use so the x8 image loads own the DMA engines at startup.

Hardware constraints baked in (probed): matmul dst partition base must be 0;
engines cannot remap partitions (only DMA/PE move data across partitions);
no divide ALU on DVE; fp32r matmul needs N>=256; DVE ops need 32-aligned
partition bases; only gpsimd DMAs may cast dtypes; DMA issue costs ~0.7us
of issuing-engine time regardless of size (so batch DMAs).
"""
import numpy as np
from contextlib import ExitStack

import concourse.bass as bass
import concourse.bacc as bacc
import concourse.tile as tile
from concourse import mybir
from concourse.bass_utils import run_bass_kernel_spmd

F32 = mybir.dt.float32
F32R = mybir.dt.float32r
F16 = mybir.dt.float16

B, C, H, W = 2, 16, 128, 128
PS = (3, 5, 7)
PADS = (1, 2, 3)
NBG, NTG = 64, 8
NCORES = 8
NBANDS = 4
BH = H // NBANDS            # 32 out rows per core
RX = 38                     # x replica rows per core
CX = 134                    # x cols with halo (128 + 6)
RA = 38                     # max att rows (32 + 2*padmax)
RAL = 40                    # R sbuf rows (fold-tree halo)
RAS = [BH + 2 * p for p in PADS]   # att rows per branch: 34, 36, 38
SEQ = (2, 1, 0)             # branch processing order: p=7, 5, 3
W2BASE = [0, 2, 5]

_CACHE = {}


def _windows(ra):
    return [(r0, min(4, ra - r0)) for r0 in range(0, ra, 4)]


def _build_A():
    nc = bacc.Bacc("TRN2", target_bir_lowering=False, debug=False,
                   num_devices=NCORES)

    d_x8bg = nc.dram_tensor("x8bg", [112, RX, CX], F16, kind="ExternalInput")
    d_x8tg = nc.dram_tensor("x8tg", [112, RX, CX], F16, kind="ExternalInput")
    d_hug = nc.dram_tensor("hug", [3, RA, W], F32, kind="ExternalInput")
    d_rdiv = nc.dram_tensor("rdiv", [96, BH, W], F16, kind="ExternalInput")
    d_ones = nc.dram_tensor("oneslhs", [73, 72], F32, kind="ExternalInput")
    d_w1all = nc.dram_tensor("w1all", [112, 1080], F16, kind="ExternalInput")
    d_selw = nc.dram_tensor("selw", [128, 3, 16], F16, kind="ExternalInput")
    d_w2bg = nc.dram_tensor("w2bg", [128, 9, 128], F16, kind="ExternalInput")
    d_w2tg = nc.dram_tensor("w2tg", [64, 3, 128], F16, kind="ExternalInput")
    d_fdiv = nc.dram_tensor("fdiv_out", [96, BH, W], F16,
                            kind="ExternalOutput")
    d_pool = nc.dram_tensor("pool_out", [32], F32, kind="ExternalOutput")

    with tile.TileContext(nc) as tc, ExitStack() as ctx:
        P = ctx.enter_context(tc.tile_pool(name="persist", bufs=1))
        pE = ctx.enter_context(tc.tile_pool(name="epool", bufs=2))
        pEtg = ctx.enter_context(tc.tile_pool(name="etg", bufs=2))
        pRcp = ctx.enter_context(tc.tile_pool(name="rcp", bufs=2))
        pS = ctx.enter_context(tc.tile_pool(name="spool", bufs=2))
        pR = ctx.enter_context(tc.tile_pool(name="rsb", bufs=4))
        pQ = ctx.enter_context(tc.tile_pool(name="qpool", bufs=2))
        pQo = ctx.enter_context(tc.tile_pool(name="qout", bufs=1))
        ps_c1bg = ctx.enter_context(
            tc.tile_pool(name="pc1bg", bufs=2, space=bass.MemorySpace.PSUM))
        ps_c1tg = ctx.enter_context(
            tc.tile_pool(name="pc1tg", bufs=1, space=bass.MemorySpace.PSUM))
        ps_den = ctx.enter_context(
            tc.tile_pool(name="pden", bufs=2, space=bass.MemorySpace.PSUM))
        ps_c2 = ctx.enter_context(
            tc.tile_pool(name="pc2", bufs=3, space=bass.MemorySpace.PSUM))

        # ---- startup-critical loads only: x8 images + first-branch weights
        x8 = {}
        w1all = P.tile([112, 1080], F16, tag="w1all")
        nc.scalar.dma_start(w1all[:], d_w1all[:])
        t = P.tile([112, RX, CX], F16, tag="x8bg")
        nc.sync.dma_start(t[:, 0:20, :], d_x8bg[:, 0:20, :])
        x8["bg"] = t
        t = P.tile([112, RX, CX], F16, tag="x8tg")
        nc.gpsimd.dma_start(t[:, 0:20, :], d_x8tg[:, 0:20, :])
        x8["tg"] = t
        nc.sync.dma_start(x8["bg"][:, 20:RX, :], d_x8bg[:, 20:RX, :])
        nc.gpsimd.dma_start(x8["tg"][:, 20:RX, :], d_x8tg[:, 20:RX, :])
        # packed col offsets: bg0,bg1,bg2 then tg0,tg1,tg2
        W1OFF = {("bg", 0): 0, ("bg", 1): 192, ("bg", 2): 512,
                 ("tg", 0): 960, ("tg", 1): 984, ("tg", 2): 1024}

        def w1ap(s, n, j):
            M = NBG if s == "bg" else NTG
            off = W1OFF[(s, n)] + j * M
            return w1all[0:16 * PS[n], off:off + M]

        ones_l = P.tile([73, 72], F32R, tag="ones")
        nc.gpsimd.dma_start(ones_l[:], d_ones[:])

        fdiv = P.tile([96, BH, W], F16, tag="fdiv")
        pacc = P.tile([96, 1], F32, tag="pacc")
        late = {}   # deferred persistent tiles: w2bg, w2tg, rdiv

        state = {}

        def load_late():
            t = P.tile([128, 9, 128], F16, tag="w2bg")
            nc.scalar.dma_start(t[:], d_w2bg[:])
            late["w2bg"] = t
            t = P.tile([64, 3, 128], F16, tag="w2tg")
            nc.scalar.dma_start(t[:], d_w2tg[:])
            late["w2tg"] = t
            t = P.tile([96, BH, W], F16, tag="rdiv")
            nc.scalar.dma_start(t[:], d_rdiv[:])
            late["rdiv"] = t
            t = P.tile([128, 3, 16], F16, tag="selw")
            nc.scalar.dma_start(t[:], d_selw[:])
            late["selw"] = t

        def conv1(k):
            n = SEQ[k]
            p, pad, ra = PS[n], PADS[n], RAS[n]
            rxo = 6 - 2 * pad
            wins = _windows(ra)
            nw = len(wins)
            E = pE.tile([73, RA, W], F32R, tag="E")
            nc.gpsimd.dma_start(E[72:73, 0:ra, :], d_hug[n:n + 1, 0:ra, :])
            Sbg = pS.tile([128, RA, 136], F16, tag="Sbg")
            Stg = pS.tile([72, RA, 144], F16, tag="Stg")
            nc.gpsimd.memset(Sbg[:, :, 0:4], 0.0)
            nc.gpsimd.memset(Sbg[:, :, 131:136], 0.0)
            nc.gpsimd.memset(Stg[:, :, 0:15], 0.0)
            nc.gpsimd.memset(Stg[:, :, 136:144], 0.0)
            st_etg = {}

            def denom_pair(w0):
                # denominator + softmax muls for windows w0, w0+1 (<=8 rows)
                r0 = wins[w0][0]
                rr8 = wins[w0][1] + (wins[w0 + 1][1] if w0 + 1 < nw else 0)
                rcp = pRcp.tile([72, 8, W], F32, tag="rcp")
                segs = [(0, wins[w0][1])]
                if w0 + 1 < nw:
                    segs.append((wins[w0][1], wins[w0 + 1][1]))
                for h, rr in segs:
                    den = ps_den.tile([72, 4, W], F32, tag="den")
                    nc.tensor.matmul(den[0:72, 0:rr, :], ones_l[:, 0:72],
                                     E[:, r0 + h:r0 + h + rr, :],
                                     start=True, stop=True)
                    nc.vector.reciprocal_approx_fast(rcp[0:72, h:h + rr, :],
                                                     den[0:72, 0:rr, :])
                nc.vector.tensor_mul(Sbg[0:64, r0:r0 + rr8, 3:131],
                                     E[0:64, r0:r0 + rr8, :],
                                     rcp[0:64, 0:rr8, :])
                nc.gpsimd.tensor_mul(Stg[64:72, r0:r0 + rr8, 8:136],
                                     E[64:72, r0:r0 + rr8, :],
                                     rcp[64:72, 0:rr8, :])

            for w, (r0, rr) in enumerate(wins):
                st = ps_c1bg.tile([64, 4, W], F32, tag="c1bg")
                for j in range(p):
                    nc.tensor.matmul(
                        st[0:64, 0:rr, :],
                        w1ap("bg", n, j),
                        x8["bg"][0:16 * p, r0 + rxo:r0 + rxo + rr,
                                 j + 3 - pad:j + 3 - pad + W],
                        start=(j == 0), stop=(j == p - 1))
                nc.scalar.activation(E[0:64, r0:r0 + rr, :], st[0:64, 0:rr, :],
                                     mybir.ActivationFunctionType.Exp)
                stg = ps_c1tg.tile([8, 4, W], F32, tag="c1tg")
                for j in range(p):
                    nc.tensor.matmul(
                        stg[0:8, 0:rr, :],
                        w1ap("tg", n, j),
                        x8["tg"][0:16 * p, r0 + rxo:r0 + rxo + rr,
                                 j + 3 - pad:j + 3 - pad + W],
                        start=(j == 0), stop=(j == p - 1))
                if w % 2 == 0:
                    st_etg[w // 2] = pEtg.tile([8, 8, W], F32R, tag="etg", name="etg")
                etg = st_etg[w // 2]
                h = 0 if w % 2 == 0 else wins[w - 1][1]
                nc.scalar.activation(etg[0:8, h:h + rr, :], stg[0:8, 0:rr, :],
                                     mybir.ActivationFunctionType.Exp)
                if w % 2 == 1 or w == nw - 1:
                    w0 = w - (w % 2)
                    hh = wins[w0][1] + (rr if w % 2 == 1 else 0)
                    nc.gpsimd.dma_start(
                        E[64:72, wins[w0][0]:wins[w0][0] + hh, :],
                        etg[0:8, 0:hh, :])
                    if w0 >= 2:
                        denom_pair(w0 - 2)
                # deferred loads ride behind the first windows
                if k == 0 and w == 2:
                    load_late()
            # remaining denominator pair (in-loop covered up to last-2)
            denom_pair((nw - 1) - ((nw - 1) % 2))
            state[k] = (Sbg, Stg)

        def replicas(k):
            # full-branch replica DMAs; emitted as late as possible so the
            # (conservative, queue-cumulative) sync-DMA dependency horizon of
            # earlier conv2 stages never includes them
            n = SEQ[k]
            p, ra = PS[n], RAS[n]
            Sbg, Stg = state[k]
            for g in range(p):
                nc.sync.dma_start(Stg[8 * g:8 * g + 8, 0:ra, 8 + g:136 + g],
                                  Stg[64:72, 0:ra, 8:136])
            nc.sync.dma_start(Sbg[64:128, 0:ra, 4:132], Sbg[0:64, 0:ra, 3:131])

        def fold_dma(k):
            # fold_y stage 1: align each group's rows with per-group DMAs
            # (only DMAs can shift rows per partition group); emitted right
            # after conv2(k) so the sync queue runs these before the next
            # branch's replicas.
            n = SEQ[k]
            p = PS[n]
            Rs = state[k]
            Q = {}
            for si in range(2):
                Q[si] = pQ.tile([128, BH, W], F16, tag="Q", name=f"Q{si}")
                for g in range(p):
                    nc.gpsimd.dma_start(Q[si][16 * g:16 * g + 16, :, :],
                                        Rs[si][16 * g:16 * g + 16, g:g + BH, :])
            state[("Q", k)] = Q

        def fold_mm_gen(k):
            # fold_y stage 2: contract the groups with a 0/1 selection matrix
            # (K=16p, M=16); yielded in steps so conv2 can interleave them.
            n = SEQ[k]
            p = PS[n]
            Q = state[("Q", k)]
            sel = late["selw"]
            for si in range(2):
                Qo = pQo.tile([16, BH, W], F16, tag="Qo", name=f"Qo{si}")
                for r0 in range(0, BH, 4):
                    rpf = ps_c2.tile([16, 4, W], F32, tag="c2", name="rpf")
                    nc.tensor.matmul(rpf[0:16, :, :], sel[0:16 * p, n, :],
                                     Q[si][0:16 * p, r0:r0 + 4, :],
                                     start=True, stop=True)
                    nc.vector.tensor_copy(Qo[0:16, r0:r0 + 4, :],
                                          rpf[0:16, :, :])
                    yield
                nc.gpsimd.dma_start(
                    fdiv[32 * n + 16 * si:32 * n + 16 * si + 16, :, :],
                    Qo[:])
            nc.vector.scalar_tensor_tensor(
                fdiv[32 * n:32 * n + 32, :, :],
                fdiv[32 * n:32 * n + 32, :, :], 0.0,
                late["rdiv"][32 * n:32 * n + 32, :, :],
                op0=mybir.AluOpType.bypass, op1=mybir.AluOpType.mult,
                accum_out=pacc[32 * n:32 * n + 32, :])
            nc.sync.dma_start(d_fdiv[32 * n:32 * n + 32, :, :],
                              fdiv[32 * n:32 * n + 32, :, :])

        def conv2(k, foldgen=None):
            n = SEQ[k]
            p, pad, ra = PS[n], PADS[n], RAS[n]
            Sbg, Stg = state[k]
            w2bg, w2tg = late["w2bg"], late["w2tg"]
            Rbg = pR.tile([128, RA, W], F16, tag="R")
            Rtg = pR.tile([128, RA, W], F16, tag="R")
            nchk = (p + 1) // 2
            for r0, rr in _windows(ra):
                rp = ps_c2.tile([128, 4, W], F32, tag="c2")
                for ci in range(nchk):
                    jj = 2 * ci
                    nc.tensor.matmul(
                        rp[:, 0:rr, :],
                        w2bg[:, W2BASE[n] + ci, :],
                        Sbg[:, r0:r0 + rr, 3 + pad - jj:3 + pad - jj + W],
                        start=(ci == 0), stop=(ci == nchk - 1))
                nc.scalar.activation(Rbg[:, r0:r0 + rr, :], rp[:, 0:rr, :],
                                     mybir.ActivationFunctionType.Copy)
                rp2 = ps_c2.tile([128, 4, W], F32, tag="c2")
                nc.tensor.matmul(rp2[0:128, 0:rr, :],
                                 w2tg[0:8 * p, n, :],
                                 Stg[0:8 * p, r0:r0 + rr, 8 + pad:8 + pad + W],
                                 start=True, stop=True)
                nc.scalar.activation(Rtg[:, r0:r0 + rr, :], rp2[:, 0:rr, :],
                                     mybir.ActivationFunctionType.Copy)
                if foldgen is not None:
                    next(foldgen, None)
                    next(foldgen, None)
            if foldgen is not None:
                for _ in foldgen:
                    pass
            state[k] = (Rbg, Rtg)

        def drain(gen):
            for _ in gen:
                pass

        # ---------------- pipelined emission ----------------
        conv1(0)
        replicas(0)
        conv1(1)
        conv2(0)
        replicas(1)
        fold_dma(0)
        conv1(2)
        conv2(1, foldgen=fold_mm_gen(0))
        replicas(2)
        fold_dma(1)
        conv2(2, foldgen=fold_mm_gen(1))
        fold_dma(2)
        drain(fold_mm_gen(2))

        # pooled partial combine -> pool_out
        pb = P.tile([32, 1], F32, tag="pb")
        pc = P.tile([32, 1], F32, tag="pc")
        nc.sync.dma_start(pb[:], pacc[32:64, :])
        nc.sync.dma_start(pc[:], pacc[64:96, :])
        pool32a = P.tile([32, 1], F32, tag="pool32a")
        pool32 = P.tile([32, 1], F32, tag="pool32")
        nc.vector.tensor_add(pool32a[:], pacc[0:32, :], pb[:])
        nc.vector.tensor_add(pool32[:], pool32a[:], pc[:])
        nc.sync.dma_start(d_pool[:], pool32[:, 0])

    nc.compile()
    return nc


def _build_B():
    nc = bacc.Bacc("TRN2", target_bir_lowering=False, debug=False,
                   num_devices=NCORES)
    d_f = nc.dram_tensor("fdiv_in", [96, BH, W], F16, kind="ExternalInput")
    d_wt = nc.dram_tensor("wt96", [96, 1], F32, kind="ExternalInput")
    d_obg = nc.dram_tensor("out_bg", [C, BH, W], F16, kind="ExternalOutput")
    d_otg = nc.dram_tensor("out_tg", [C, BH, W], F16, kind="ExternalOutput")

    with tile.TileContext(nc) as tc, ExitStack() as ctx:
        Q = ctx.enter_context(tc.tile_pool(name="q", bufs=1))
        fdv = Q.tile([96, BH, W], F16, tag="fdv")
        wt = Q.tile([96, 1], F32, tag="wt")
        nc.sync.dma_start(fdv[0:48, :, :], d_f[0:48, :, :])
        nc.scalar.dma_start(fdv[48:96, :, :], d_f[48:96, :, :])
        nc.sync.dma_start(wt[:], d_wt[:])
        nc.vector.tensor_scalar_mul(fdv[:], fdv[:], wt[:])
        gb = Q.tile([32, BH, W], F16, tag="gb")
        gc = Q.tile([32, BH, W], F16, tag="gc")
        nc.sync.dma_start(gb[:], fdv[32:64, :, :])
        nc.scalar.dma_start(gc[:], fdv[64:96, :, :])
        nc.vector.tensor_add(fdv[0:32, :, :], fdv[0:32, :, :], gb[:])
        nc.vector.tensor_add(fdv[0:32, :, :], fdv[0:32, :, :], gc[:])
        nc.sync.dma_start(d_obg[:], fdv[0:16, :, :])
        nc.sync.dma_start(d_otg[:], fdv[16:32, :, :])

    nc.compile()
    return nc


# ======================= host-side prep =======================

def _prep_core(inputs, b, k):
    y0 = BH * k
    m = {}
    for s, key in (("bg", "bg"), ("tg", "tg")):
        x = np.asarray(inputs[key])[b]          # [C, H, W]
        x8 = np.zeros((7, C, RX, CX), np.float32)
        for g in range(7):
            lo = y0 - 6 + g
            hi = lo + RX
            slo, shi = max(lo, 0), min(hi, H)
            if slo < shi:
                x8[g, :, slo - lo:shi - lo, 3:131] = x[:, slo:shi, :]
        m[f"x8{s}"] = x8.reshape(112, RX, CX).astype(np.float16)

    hug = np.zeros((3, RA, W), np.float32)
    for n, pad in enumerate(PADS):
        for r in range(RA):
            y = y0 - pad + r
            if not (0 <= y < H):
                hug[n, r, :] = 1e30
    m["hug"] = hug

    rdiv = np.zeros((96, BH, W), np.float32)
    for n, pad in enumerate(PADS):
        yy = np.arange(H)
        rc = np.minimum(yy, pad) + np.minimum(H - 1 - yy, pad) + 1.0
        cc = np.minimum(yy[:W], pad) + np.minimum(W - 1 - yy[:W], pad) + 1.0
        div = np.outer(rc[y0:y0 + BH], cc) + 1e-8
        r = (1.0 / div).astype(np.float32)
        for si in range(2):
            base = 32 * n + 16 * si
            rdiv[base:base + 16] = r[None, :, :]
    m["rdiv"] = rdiv.astype(np.float16)

    ones = np.zeros((73, 72), np.float32)
    ones[0:64, 0:64] = 1.0
    ones[64:72, 64:72] = 1.0
    ones[72, :] = 1.0
    m["oneslhs"] = ones

    w1all = np.zeros((112, 1080), np.float32)
    w1off = {("bg", 0): 0, ("bg", 1): 192, ("bg", 2): 512,
             ("tg", 0): 960, ("tg", 1): 984, ("tg", 2): 1024}
    for s, M, nmem in (("bg", NBG, "bg_mem"), ("tg", NTG, "tg_mem")):
        for n, p in enumerate(PS):
            mem = np.asarray(inputs[f"{nmem}{n}"])          # [M, C*p*p]
            temp = float(np.asarray(inputs[f"{s}_temp{n}"])[0])
            D = C * p * p
            arr = mem.reshape(M, C, p, p)
            w1 = arr.transpose(2, 1, 3, 0).reshape(p * C, p * M)
            off = w1off[(s, n)]
            w1all[0:16 * p, off:off + p * M] = w1 * (temp / np.sqrt(D))
    m["w1all"] = w1all.astype(np.float16)

    # fold consumes group q at row shift +q where q = 2*pad - i
    w2bg = np.zeros((2, NBG, 9, 8, 16), np.float32)
    for n, p in enumerate(PS):
        pad = PADS[n]
        arr = np.asarray(inputs[f"bg_mem{n}"]).reshape(NBG, C, p, p)
        for ci in range((p + 1) // 2):
            for g in range(2):
                j = 2 * ci + g
                if j < p:
                    for i in range(p):
                        w2bg[g, :, W2BASE[n] + ci, 2 * pad - i, :] = \
                            arr[:, :, i, j]
    m["w2bg"] = w2bg.reshape(128, 9, 128).astype(np.float16)

    w2tg = np.zeros((8, NTG, 3, 8, 16), np.float32)
    for n, p in enumerate(PS):
        pad = PADS[n]
        arr = np.asarray(inputs[f"tg_mem{n}"]).reshape(NTG, C, p, p)
        for g in range(p):
            for i in range(p):
                w2tg[g, :, n, 2 * pad - i, :] = arr[:, :, i, g]
    m["w2tg"] = w2tg.reshape(64, 3, 128).astype(np.float16)

    selw = np.zeros((128, 3, 16), np.float32)
    for n, p in enumerate(PS):
        for g in range(p):
            for c in range(16):
                selw[16 * g + c, n, c] = 1.0
    m["selw"] = selw.astype(np.float16)
    return m


def _host_mlp(inputs, poolsum):
    """Per batch: pooled -> relu MLP -> softmax over scales -> wt96."""
    wt96 = np.zeros((96, 1), np.float32)
    for si, s in enumerate(("bg", "tg")):
        pooled = poolsum[16 * si:16 * si + 16] / (H * W)
        w1 = np.asarray(inputs[f"{s}_fc1_w"], np.float64)
        b1 = np.asarray(inputs[f"{s}_fc1_b"], np.float64)
        w2 = np.asarray(inputs[f"{s}_fc2_w"], np.float64)
        b2 = np.asarray(inputs[f"{s}_fc2_b"], np.float64)
        hdn = np.maximum(w1 @ pooled + b1, 0.0)
        logits = (w2 @ hdn + b2).reshape(3, 16)
        e = np.exp(logits - logits.max(axis=0, keepdims=True))
        wt = e / e.sum(axis=0, keepdims=True)
        for n in range(3):
            wt96[32 * n + 16 * si:32 * n + 16 * si + 16, 0] = wt[n]
    return wt96


def kernel(**inputs):
    if "ncA" not in _CACHE:
        _CACHE["ncA"] = _build_A()
        _CACHE["ncB"] = _build_B()

    in_maps = []
    for core in range(NCORES):
        b, k = divmod(core, NBANDS)
        in_maps.append(_prep_core(inputs, b, k))

    resA = run_bass_kernel_spmd(_CACHE["ncA"], in_maps, list(range(NCORES)))

    # host glue: reduce pooled partials within each batch's 4-band group,
    # then the tiny fusion MLP (exact, fp64)
    wt = {}
    for b in range(B):
        poolsum = np.sum([resA.results[b * NBANDS + k]["pool_out"]
                          for k in range(NBANDS)], axis=0).astype(np.float64)
        wt[b] = _host_mlp(inputs, poolsum)

    in_mapsB = []
    for core in range(NCORES):
        b, k = divmod(core, NBANDS)
        in_mapsB.append({
            "fdiv_in": resA.results[core]["fdiv_out"],
            "wt96": wt[b],
        })
    resB = run_bass_kernel_spmd(_CACHE["ncB"], in_mapsB, list(range(NCORES)))

    f_bc = np.zeros((B, C, H, W), np.float32)
    f_tg = np.zeros((B, C, H, W), np.float32)
    for core in range(NCORES):
        b, k = divmod(core, NBANDS)
        y0 = BH * k
        f_bc[b, :, y0:y0 + BH, :] = resB.results[core]["out_bg"].astype(np.float32)
        f_tg[b, :, y0:y0 + BH, :] = resB.results[core]["out_tg"].astype(np.float32)
    return (f_bc, f_tg)


# revision 22
# speedup vs baseline: 1.2346x; 1.0323x over previous
"""DualMemorySystem Trainium2 kernel — 8-core SPMD (batch x 4 row-bands).

Per core: one (b, 32-row out band). Convolution form of unfold/attention/fold:
  sim = conv(x, mem)      -> p matmuls per 4-row window, K=(kernel-row, c), fp16
  att = softmax_m(sim)    -> exp (ACT), ones-matmul partition sum,
                             reciprocal_approx (DVE) + multiplies (DVE+GpSimd)
  R_i = conv_x(att, mem)  -> matmuls over col-shifted att replicas, fp16
  out = fold_y(R)         -> log-tree of shifted adds (in-place), partition
                             moves via SBUF->SBUF DMA
  fusion: pooled partials via STT accum -> host MLP between launches ->
          tiny phase-B kernel applies softmax weights + combines.

Software-pipelined emission: branch order (p=7, p=5, p=3); per branch the
PE stream is conv1(k) [denominator matmuls trail at a 2-window lag], then
conv2(k-1), so the PE never waits on the current branch's softmax chain.
Double-buffered PSUM pools (8 banks exactly). DMAs ride only the sync and
gpsimd queues mid-kernel; large weight loads are deferred to just before
first use so the x8 image loads own the DMA engines at startup.

Hardware constraints baked in (probed): matmul dst partition base must be 0;
engines cannot remap partitions (only DMA/PE move data across partitions);
no divide ALU on DVE; fp32r matmul needs N>=256; DVE ops need 32-aligned
partition bases; only gpsimd DMAs may cast dtypes; DMA issue costs ~0.7us
of issuing-engine time regardless of size (so batch DMAs).
"""
import numpy as np
from contextlib import ExitStack

import concourse.bass as bass
import concourse.bacc as bacc
import concourse.tile as tile
from concourse import mybir
from concourse.bass_utils import run_bass_kernel_spmd

F32 = mybir.dt.float32
F32R = mybir.dt.float32r
F16 = mybir.dt.float16

B, C, H, W = 2, 16, 128, 128
PS = (3, 5, 7)
PADS = (1, 2, 3)
NBG, NTG = 64, 8
NCORES = 8
NBANDS = 4
BH = H // NBANDS            # 32 out rows per core
RX = 38                     # x replica rows per core
CX = 134                    # x cols with halo (128 + 6)
RA = 38                     # max att rows (32 + 2*padmax)
RAL = 40                    # R sbuf rows (fold-tree halo)
RAS = [BH + 2 * p for p in PADS]   # att rows per branch: 34, 36, 38
SEQ = (2, 1, 0)             # branch processing order: p=7, 5, 3
W2BASE = [0, 2, 5]

_CACHE = {}


def _windows(ra):
    return [(r0, min(4, ra - r0)) for r0 in range(0, ra, 4)]


def _build_A():
    nc = bacc.Bacc("TRN2", target_bir_lowering=False, debug=False,
                   num_devices=NCORES)

    d_x8bg = nc.dram_tensor("x8bg", [112, RX, CX], F16, kind="ExternalInput")
    d_x8tg = nc.dram_tensor("x8tg", [112, RX, CX], F16, kind="ExternalInput")
    d_hug = nc.dram_tensor("hug", [3, RA, W], F32, kind="ExternalInput")
    d_rdiv = nc.dram_tensor("rdiv", [96, BH, W], F16, kind="ExternalInput")
    d_ones = nc.dram_tensor("oneslhs", [73, 72], F32, kind="ExternalInput")
    d_w1all = nc.dram_tensor("w1all", [112, 1080], F16, kind="ExternalInput")
    d_selw = nc.dram_tensor("selw", [128, 3, 16], F16, kind="ExternalInput")
    d_w2bg = nc.dram_tensor("w2bg", [128, 9, 128], F16, kind="ExternalInput")
    d_w2tg = nc.dram_tensor("w2tg", [64, 3, 128], F16, kind="ExternalInput")
    d_fdiv = nc.dram_tensor("fdiv_out", [96, BH, W], F16,
                            kind="ExternalOutput")
    d_pool = nc.dram_tensor("pool_out", [32], F32, kind="ExternalOutput")

    with tile.TileContext(nc) as tc, ExitStack() as ctx:
        P = ctx.enter_context(tc.tile_pool(name="persist", bufs=1))
        pE = ctx.enter_context(tc.tile_pool(name="epool", bufs=2))
        pEtg = ctx.enter_context(tc.tile_pool(name="etg", bufs=2))
        pRcp = ctx.enter_context(tc.tile_pool(name="rcp", bufs=2))
        pS = ctx.enter_context(tc.tile_pool(name="spool", bufs=2))
        pR = ctx.enter_context(tc.tile_pool(name="rsb", bufs=4))
        pQ = ctx.enter_context(tc.tile_pool(name="qpool", bufs=2))
        pQo = ctx.enter_context(tc.tile_pool(name="qout", bufs=1))
        ps_c1bg = ctx.enter_context(
            tc.tile_pool(name="pc1bg", bufs=2, space=bass.MemorySpace.PSUM))
        ps_c1tg = ctx.enter_context(
            tc.tile_pool(name="pc1tg", bufs=1, space=bass.MemorySpace.PSUM))
        ps_den = ctx.enter_context(
            tc.tile_pool(name="pden", bufs=2, space=bass.MemorySpace.PSUM))
        ps_c2 = ctx.enter_context(
            tc.tile_pool(name="pc2", bufs=3, space=bass.MemorySpace.PSUM))

        # ---- startup-critical loads only: x8 images + first-branch weights
        x8 = {}
        w1all = P.tile([112, 1080], F16, tag="w1all")
        nc.scalar.dma_start(w1all[:, 512:960], d_w1all[:, 512:960])
        nc.scalar.dma_start(w1all[:, 1024:1080], d_w1all[:, 1024:1080])
        t = P.tile([112, RX, CX], F16, tag="x8bg")
        nc.sync.dma_start(t[:, 0:20, :], d_x8bg[:, 0:20, :])
        x8["bg"] = t
        t = P.tile([112, RX, CX], F16, tag="x8tg")
        nc.gpsimd.dma_start(t[:, 0:20, :], d_x8tg[:, 0:20, :])
        x8["tg"] = t
        nc.sync.dma_start(x8["bg"][:, 20:RX, :], d_x8bg[:, 20:RX, :])
        nc.gpsimd.dma_start(x8["tg"][:, 20:RX, :], d_x8tg[:, 20:RX, :])
        nc.scalar.dma_start(w1all[:, 0:512], d_w1all[:, 0:512])
        nc.scalar.dma_start(w1all[:, 960:1024], d_w1all[:, 960:1024])
        # packed col offsets: bg0,bg1,bg2 then tg0,tg1,tg2
        W1OFF = {("bg", 0): 0, ("bg", 1): 192, ("bg", 2): 512,
                 ("tg", 0): 960, ("tg", 1): 984, ("tg", 2): 1024}

        def w1ap(s, n, j):
            M = NBG if s == "bg" else NTG
            off = W1OFF[(s, n)] + j * M
            return w1all[0:16 * PS[n], off:off + M]

        ones_l = P.tile([73, 72], F32R, tag="ones")
        nc.gpsimd.dma_start(ones_l[:], d_ones[:])

        fdiv = P.tile([96, BH, W], F16, tag="fdiv")
        pacc = P.tile([96, 1], F32, tag="pacc")
        late = {}   # deferred persistent tiles: w2bg, w2tg, rdiv

        state = {}

        def load_late():
            t = P.tile([128, 9, 128], F16, tag="w2bg")
            nc.scalar.dma_start(t[:], d_w2bg[:])
            late["w2bg"] = t
            t = P.tile([64, 3, 128], F16, tag="w2tg")
            nc.scalar.dma_start(t[:], d_w2tg[:])
            late["w2tg"] = t
            t = P.tile([96, BH, W], F16, tag="rdiv")
            nc.scalar.dma_start(t[:], d_rdiv[:])
            late["rdiv"] = t
            t = P.tile([128, 3, 16], F16, tag="selw")
            nc.scalar.dma_start(t[:], d_selw[:])
            late["selw"] = t

        def conv1(k):
            n = SEQ[k]
            p, pad, ra = PS[n], PADS[n], RAS[n]
            rxo = 6 - 2 * pad
            wins = _windows(ra)
            nw = len(wins)
            E = pE.tile([73, RA, W], F32R, tag="E")
            nc.gpsimd.dma_start(E[72:73, 0:ra, :], d_hug[n:n + 1, 0:ra, :])
            Sbg = pS.tile([128, RA, 136], F16, tag="Sbg")
            Stg = pS.tile([72, RA, 144], F16, tag="Stg")
            nc.gpsimd.memset(Sbg[:, :, 0:4], 0.0)
            nc.gpsimd.memset(Sbg[:, :, 131:136], 0.0)
            nc.gpsimd.memset(Stg[:, :, 0:15], 0.0)
            nc.gpsimd.memset(Stg[:, :, 136:144], 0.0)
            st_etg = {}

            def denom_pair(w0):
                # denominator + softmax muls for windows w0, w0+1 (<=8 rows)
                r0 = wins[w0][0]
                rr8 = wins[w0][1] + (wins[w0 + 1][1] if w0 + 1 < nw else 0)
                rcp = pRcp.tile([72, 8, W], F32, tag="rcp")
                segs = [(0, wins[w0][1])]
                if w0 + 1 < nw:
                    segs.append((wins[w0][1], wins[w0 + 1][1]))
                for h, rr in segs:
                    den = ps_den.tile([72, 4, W], F32, tag="den")
                    nc.tensor.matmul(den[0:72, 0:rr, :], ones_l[:, 0:72],
                                     E[:, r0 + h:r0 + h + rr, :],
                                     start=True, stop=True)
                    nc.vector.reciprocal_approx_fast(rcp[0:72, h:h + rr, :],
                                                     den[0:72, 0:rr, :])
                nc.vector.tensor_mul(Sbg[0:64, r0:r0 + rr8, 3:131],
                                     E[0:64, r0:r0 + rr8, :],
                                     rcp[0:64, 0:rr8, :])
                nc.gpsimd.tensor_mul(Stg[64:72, r0:r0 + rr8, 8:136],
                                     E[64:72, r0:r0 + rr8, :],
                                     rcp[64:72, 0:rr8, :])

            for w, (r0, rr) in enumerate(wins):
                st = ps_c1bg.tile([64, 4, W], F32, tag="c1bg")
                for j in range(p):
                    nc.tensor.matmul(
                        st[0:64, 0:rr, :],
                        w1ap("bg", n, j),
                        x8["bg"][0:16 * p, r0 + rxo:r0 + rxo + rr,
                                 j + 3 - pad:j + 3 - pad + W],
                        start=(j == 0), stop=(j == p - 1))
                nc.scalar.activation(E[0:64, r0:r0 + rr, :], st[0:64, 0:rr, :],
                                     mybir.ActivationFunctionType.Exp)
                stg = ps_c1tg.tile([8, 4, W], F32, tag="c1tg")
                for j in range(p):
                    nc.tensor.matmul(
                        stg[0:8, 0:rr, :],
                        w1ap("tg", n, j),
                        x8["tg"][0:16 * p, r0 + rxo:r0 + rxo + rr,
                                 j + 3 - pad:j + 3 - pad + W],
                        start=(j == 0), stop=(j == p - 1))
                if w % 2 == 0:
                    st_etg[w // 2] = pEtg.tile([8, 8, W], F32R, tag="etg", name="etg")
                etg = st_etg[w // 2]
                h = 0 if w % 2 == 0 else wins[w - 1][1]
                nc.scalar.activation(etg[0:8, h:h + rr, :], stg[0:8, 0:rr, :],
                                     mybir.ActivationFunctionType.Exp)
                if w % 2 == 1 or w == nw - 1:
                    w0 = w - (w % 2)
                    hh = wins[w0][1] + (rr if w % 2 == 1 else 0)
                    nc.gpsimd.dma_start(
                        E[64:72, wins[w0][0]:wins[w0][0] + hh, :],
                        etg[0:8, 0:hh, :])
                    if w0 >= 2:
                        denom_pair(w0 - 2)
                # deferred loads ride behind the first windows
                if k == 0 and w == 2:
                    load_late()
            # remaining denominator pair (in-loop covered up to last-2)
            denom_pair((nw - 1) - ((nw - 1) % 2))
            state[k] = (Sbg, Stg)

        def replicas(k):
            # full-branch replica DMAs; emitted as late as possible so the
            # (conservative, queue-cumulative) sync-DMA dependency horizon of
            # earlier conv2 stages never includes them
            n = SEQ[k]
            p, ra = PS[n], RAS[n]
            Sbg, Stg = state[k]
            for g in range(p):
                nc.sync.dma_start(Stg[8 * g:8 * g + 8, 0:ra, 8 + g:136 + g],
                                  Stg[64:72, 0:ra, 8:136])
            nc.sync.dma_start(Sbg[64:128, 0:ra, 4:132], Sbg[0:64, 0:ra, 3:131])

        def fold_dma(k):
            # fold_y stage 1: align each group's rows with per-group DMAs
            # (only DMAs can shift rows per partition group); emitted right
            # after conv2(k) so the sync queue runs these before the next
            # branch's replicas.
            n = SEQ[k]
            p = PS[n]
            Rs = state[k]
            Q = {}
            for si in range(2):
                Q[si] = pQ.tile([128, BH, W], F16, tag="Q", name=f"Q{si}")
                for g in range(p):
                    nc.gpsimd.dma_start(Q[si][16 * g:16 * g + 16, :, :],
                                        Rs[si][16 * g:16 * g + 16, g:g + BH, :])
            state[("Q", k)] = Q

        def fold_mm_gen(k):
            # fold_y stage 2: contract the groups with a 0/1 selection matrix
            # (K=16p, M=16); yielded in steps so conv2 can interleave them.
            n = SEQ[k]
            p = PS[n]
            Q = state[("Q", k)]
            sel = late["selw"]
            for si in range(2):
                Qo = pQo.tile([16, BH, W], F16, tag="Qo", name=f"Qo{si}")
                for r0 in range(0, BH, 4):
                    rpf = ps_c2.tile([16, 4, W], F32, tag="c2", name="rpf")
                    nc.tensor.matmul(rpf[0:16, :, :], sel[0:16 * p, n, :],
                                     Q[si][0:16 * p, r0:r0 + 4, :],
                                     start=True, stop=True)
                    nc.scalar.activation(Qo[0:16, r0:r0 + 4, :],
                                          rpf[0:16, :, :],
                                          mybir.ActivationFunctionType.Copy)
                    yield
                nc.gpsimd.dma_start(
                    fdiv[32 * n + 16 * si:32 * n + 16 * si + 16, :, :],
                    Qo[:])
            nc.vector.scalar_tensor_tensor(
                fdiv[32 * n:32 * n + 32, :, :],
                fdiv[32 * n:32 * n + 32, :, :], 0.0,
                late["rdiv"][32 * n:32 * n + 32, :, :],
                op0=mybir.AluOpType.bypass, op1=mybir.AluOpType.mult,
                accum_out=pacc[32 * n:32 * n + 32, :])
            nc.sync.dma_start(d_fdiv[32 * n:32 * n + 32, :, :],
                              fdiv[32 * n:32 * n + 32, :, :])

        def conv2(k, foldgen=None):
            n = SEQ[k]
            p, pad, ra = PS[n], PADS[n], RAS[n]
            Sbg, Stg = state[k]
            w2bg, w2tg = late["w2bg"], late["w2tg"]
            Rbg = pR.tile([128, RA, W], F16, tag="R")
            Rtg = pR.tile([128, RA, W], F16, tag="R")
            nchk = (p + 1) // 2
            for r0, rr in _windows(ra):
                rp = ps_c2.tile([128, 4, W], F32, tag="c2")
                for ci in range(nchk):
                    jj = 2 * ci
                    nc.tensor.matmul(
                        rp[:, 0:rr, :],
                        w2bg[:, W2BASE[n] + ci, :],
                        Sbg[:, r0:r0 + rr, 3 + pad - jj:3 + pad - jj + W],
                        start=(ci == 0), stop=(ci == nchk - 1))
                nc.scalar.activation(Rbg[:, r0:r0 + rr, :], rp[:, 0:rr, :],
                                     mybir.ActivationFunctionType.Copy)
                rp2 = ps_c2.tile([128, 4, W], F32, tag="c2")
                nc.tensor.matmul(rp2[0:128, 0:rr, :],
                                 w2tg[0:8 * p, n, :],
                                 Stg[0:8 * p, r0:r0 + rr, 8 + pad:8 + pad + W],
                                 start=True, stop=True)
                nc.scalar.activation(Rtg[:, r0:r0 + rr, :], rp2[:, 0:rr, :],
                                     mybir.ActivationFunctionType.Copy)
                if foldgen is not None:
                    next(foldgen, None)
                    next(foldgen, None)
            if foldgen is not None:
                for _ in foldgen:
                    pass
            state[k] = (Rbg, Rtg)

        def drain(gen):
            for _ in gen:
                pass

        # ---------------- pipelined emission ----------------
        conv1(0)
        replicas(0)
        conv1(1)
        conv2(0)
        replicas(1)
        fold_dma(0)
        conv1(2)
        conv2(1, foldgen=fold_mm_gen(0))
        replicas(2)
        fold_dma(1)
        conv2(2, foldgen=fold_mm_gen(1))
        fold_dma(2)
        drain(fold_mm_gen(2))

        # pooled partial combine -> pool_out
        pb = P.tile([32, 1], F32, tag="pb")
        pc = P.tile([32, 1], F32, tag="pc")
        nc.sync.dma_start(pb[:], pacc[32:64, :])
        nc.sync.dma_start(pc[:], pacc[64:96, :])
        pool32a = P.tile([32, 1], F32, tag="pool32a")
        pool32 = P.tile([32, 1], F32, tag="pool32")
        nc.vector.tensor_add(pool32a[:], pacc[0:32, :], pb[:])
        nc.vector.tensor_add(pool32[:], pool32a[:], pc[:])
        nc.sync.dma_start(d_pool[:], pool32[:, 0])

    nc.compile()
    return nc


def _build_B():
    nc = bacc.Bacc("TRN2", target_bir_lowering=False, debug=False,
                   num_devices=NCORES)
    d_f = nc.dram_tensor("fdiv_in", [96, BH, W], F16, kind="ExternalInput")
    d_wt = nc.dram_tensor("wt96", [96, 1], F32, kind="ExternalInput")
    d_obg = nc.dram_tensor("out_bg", [C, BH, W], F16, kind="ExternalOutput")
    d_otg = nc.dram_tensor("out_tg", [C, BH, W], F16, kind="ExternalOutput")

    with tile.TileContext(nc) as tc, ExitStack() as ctx:
        Q = ctx.enter_context(tc.tile_pool(name="q", bufs=1))
        fdv = Q.tile([96, BH, W], F16, tag="fdv")
        wt = Q.tile([96, 1], F32, tag="wt")
        gb = Q.tile([32, BH, W], F16, tag="gb")
        gc = Q.tile([32, BH, W], F16, tag="gc")
        nc.sync.dma_start(wt[:], d_wt[:])
        # row-halved software pipeline: load / scale / gather / add / store
        eng = (nc.sync, nc.scalar)
        for h in range(2):
            r = slice(16 * h, 16 * h + 16)
            eng[h].dma_start(fdv[:, r, :], d_f[:, r, :])
        for h in range(2):
            r = slice(16 * h, 16 * h + 16)
            nc.vector.tensor_scalar_mul(fdv[:, r, :], fdv[:, r, :], wt[:])
            eng[h].dma_start(gb[:, r, :], fdv[32:64, r, :])
            eng[1 - h].dma_start(gc[:, r, :], fdv[64:96, r, :])
            nc.vector.tensor_add(fdv[0:32, r, :], fdv[0:32, r, :],
                                 gb[:, r, :])
            nc.vector.tensor_add(fdv[0:32, r, :], fdv[0:32, r, :],
                                 gc[:, r, :])
            eng[h].dma_start(d_obg[:, r, :], fdv[0:16, r, :])
            eng[1 - h].dma_start(d_otg[:, r, :], fdv[16:32, r, :])

    nc.compile()
    return nc


# ======================= host-side prep =======================

def _prep_core(inputs, b, k):
    y0 = BH * k
    m = {}
    for s, key in (("bg", "bg"), ("tg", "tg")):
        x = np.asarray(inputs[key])[b]          # [C, H, W]
        x8 = np.zeros((7, C, RX, CX), np.float32)
        for g in range(7):
            lo = y0 - 6 + g
            hi = lo + RX
            slo, shi = max(lo, 0), min(hi, H)
            if slo < shi:
                x8[g, :, slo - lo:shi - lo, 3:131] = x[:, slo:shi, :]
        m[f"x8{s}"] = x8.reshape(112, RX, CX).astype(np.float16)

    hug = np.zeros((3, RA, W), np.float32)
    for n, pad in enumerate(PADS):
        for r in range(RA):
            y = y0 - pad + r
            if not (0 <= y < H):
                hug[n, r, :] = 1e30
    m["hug"] = hug

    rdiv = np.zeros((96, BH, W), np.float32)
    for n, pad in enumerate(PADS):
        yy = np.arange(H)
        rc = np.minimum(yy, pad) + np.minimum(H - 1 - yy, pad) + 1.0
        cc = np.minimum(yy[:W], pad) + np.minimum(W - 1 - yy[:W], pad) + 1.0
        div = np.outer(rc[y0:y0 + BH], cc) + 1e-8
        r = (1.0 / div).astype(np.float32)
        for si in range(2):
            base = 32 * n + 16 * si
            rdiv[base:base + 16] = r[None, :, :]
    m["rdiv"] = rdiv.astype(np.float16)

    ones = np.zeros((73, 72), np.float32)
    ones[0:64, 0:64] = 1.0
    ones[64:72, 64:72] = 1.0
    ones[72, :] = 1.0
    m["oneslhs"] = ones

    w1all = np.zeros((112, 1080), np.float32)
    w1off = {("bg", 0): 0, ("bg", 1): 192, ("bg", 2): 512,
             ("tg", 0): 960, ("tg", 1): 984, ("tg", 2): 1024}
    for s, M, nmem in (("bg", NBG, "bg_mem"), ("tg", NTG, "tg_mem")):
        for n, p in enumerate(PS):
            mem = np.asarray(inputs[f"{nmem}{n}"])          # [M, C*p*p]
            temp = float(np.asarray(inputs[f"{s}_temp{n}"])[0])
            D = C * p * p
            arr = mem.reshape(M, C, p, p)
            w1 = arr.transpose(2, 1, 3, 0).reshape(p * C, p * M)
            off = w1off[(s, n)]
            w1all[0:16 * p, off:off + p * M] = w1 * (temp / np.sqrt(D))
    m["w1all"] = w1all.astype(np.float16)

    # fold consumes group q at row shift +q where q = 2*pad - i
    w2bg = np.zeros((2, NBG, 9, 8, 16), np.float32)
    for n, p in enumerate(PS):
        pad = PADS[n]
        arr = np.asarray(inputs[f"bg_mem{n}"]).reshape(NBG, C, p, p)
        for ci in range((p + 1) // 2):
            for g in range(2):
                j = 2 * ci + g
                if j < p:
                    for i in range(p):
                        w2bg[g, :, W2BASE[n] + ci, 2 * pad - i, :] = \
                            arr[:, :, i, j]
    m["w2bg"] = w2bg.reshape(128, 9, 128).astype(np.float16)

    w2tg = np.zeros((8, NTG, 3, 8, 16), np.float32)
    for n, p in enumerate(PS):
        pad = PADS[n]
        arr = np.asarray(inputs[f"tg_mem{n}"]).reshape(NTG, C, p, p)
        for g in range(p):
            for i in range(p):
                w2tg[g, :, n, 2 * pad - i, :] = arr[:, :, i, g]
    m["w2tg"] = w2tg.reshape(64, 3, 128).astype(np.float16)

    selw = np.zeros((128, 3, 16), np.float32)
    for n, p in enumerate(PS):
        for g in range(p):
            for c in range(16):
                selw[16 * g + c, n, c] = 1.0
    m["selw"] = selw.astype(np.float16)
    return m


def _host_mlp(inputs, poolsum):
    """Per batch: pooled -> relu MLP -> softmax over scales -> wt96."""
    wt96 = np.zeros((96, 1), np.float32)
    for si, s in enumerate(("bg", "tg")):
        pooled = poolsum[16 * si:16 * si + 16] / (H * W)
        w1 = np.asarray(inputs[f"{s}_fc1_w"], np.float64)
        b1 = np.asarray(inputs[f"{s}_fc1_b"], np.float64)
        w2 = np.asarray(inputs[f"{s}_fc2_w"], np.float64)
        b2 = np.asarray(inputs[f"{s}_fc2_b"], np.float64)
        hdn = np.maximum(w1 @ pooled + b1, 0.0)
        logits = (w2 @ hdn + b2).reshape(3, 16)
        e = np.exp(logits - logits.max(axis=0, keepdims=True))
        wt = e / e.sum(axis=0, keepdims=True)
        for n in range(3):
            wt96[32 * n + 16 * si:32 * n + 16 * si + 16, 0] = wt[n]
    return wt96


def kernel(**inputs):
    if "ncA" not in _CACHE:
        _CACHE["ncA"] = _build_A()
        _CACHE["ncB"] = _build_B()

    in_maps = []
    for core in range(NCORES):
        b, k = divmod(core, NBANDS)
        in_maps.append(_prep_core(inputs, b, k))

    resA = run_bass_kernel_spmd(_CACHE["ncA"], in_maps, list(range(NCORES)))

    # host glue: reduce pooled partials within each batch's 4-band group,
    # then the tiny fusion MLP (exact, fp64)
    wt = {}
    for b in range(B):
        poolsum = np.sum([resA.results[b * NBANDS + k]["pool_out"]
                          for k in range(NBANDS)], axis=0).astype(np.float64)
        wt[b] = _host_mlp(inputs, poolsum)

    in_mapsB = []
    for core in range(NCORES):
        b, k = divmod(core, NBANDS)
        in_mapsB.append({
            "fdiv_in": resA.results[core]["fdiv_out"],
            "wt96": wt[b],
        })
    resB = run_bass_kernel_spmd(_CACHE["ncB"], in_mapsB, list(range(NCORES)))

    f_bc = np.zeros((B, C, H, W), np.float32)
    f_tg = np.zeros((B, C, H, W), np.float32)
    for core in range(NCORES):
        b, k = divmod(core, NBANDS)
        y0 = BH * k
        f_bc[b, :, y0:y0 + BH, :] = resB.results[core]["out_bg"].astype(np.float32)
        f_tg[b, :, y0:y0 + BH, :] = resB.results[core]["out_tg"].astype(np.float32)
    return (f_bc, f_tg)
